# revision 72
# baseline (speedup 1.0000x reference)
"""Trainium2 Bass kernel for multi-head causal attention with RoPE.

Model (per reference):
  B=2, S=2048, D=4096, H=32 heads, HD=128.
  out = softmax(rope(x@wq) @ rope(x@wk)^T / sqrt(HD) + mask) @ (x@wv) @ wo

Sharding: tensor-parallel over heads. Core c in 0..7 owns heads 4c..4c+3:
wq/wk/wv column-sharded, wo row-sharded; each core produces a full-shape
partial output and the host sums the 8 partials (the all-reduce).

Precision: all projections and attention matmuls run fp16 (same 1 cyc/row
as fp32r on the PE but half the DMA traffic, and no 4x penalty on
sub-256-col tiles, so the triangle-narrowed diagonals are free). PSUM
accumulation stays fp32; rope combines in fp32 and rounds once to fp16 at
the spill; the O-projection (hoT @ wo) stays fp32r. Measured rel err
1.72e-2 vs the 2e-2 gate (deterministic: fixed seed).

Causal fast path — 5 overlapped windows keeping the PE stall-free:
  W1  QK projections (512-token chunks, 8 PSUM banks), RoPE epilogue with
      PSUM drains split across ACT+DVE, fp16 partition swaps on the SWDGE
      queue, batched fp16 spills. Weights trickle in 2-dk batches one
      iteration ahead; xt tiles prefetch across chunk boundaries. The rope
      pool is allocated FIRST so the W2 pools land in w1/xt1's address
      range, whose last readers are the final matmuls — not the epilogue
      spills — ungating W2's loads ~20us earlier.
  W2  V projection batch 0 (512-token chunks, 2x4 PSUM banks); first
      attention loads and W3 x prefetches mid-window.
  W3  V projection batch 1 interleaved with attention(b0); wo loads
      spread across the window on the scalar queue.
  W4  attention(b1) interleaved with O-projection(b0), two o_pieces after
      every unit.
  W5  O-projection(b1), three-way PSUM rotation, split tail stores.
Attention per (h,qc) unit: fp16 scores with triangle-narrowed diagonal
tiles, exp on ACT to fp16, PV accumulated in PSUM, software-pipelined
three deep (scores kt+1..kt+3 issue before PV kt) with the trailing PVs
emitted by the caller AFTER the next block of independent matmuls (the
PE's in-order queue would otherwise head-of-line stall on the last exps).
Softmax denominator: fp32 DVE accumulation of ex, reduced across
partitions by gpsimd partition_all_reduce (replaces a 512-col PE
ones-matmul per unit), reciprocal+mul on DVE.
Queue plan: sync = JIT x loads only; scalar = weights/spills/attention
loads; SWDGE (gpsimd) = wait-free loads (wk, rope consts, qT reloads,
swaps) — a waiting SWDGE/HWDGE dma head-of-line blocks its engine's whole
sequencer, and HWDGE descriptor generation (~0.63us/DMA) is serialized
across the scalar+sync queues, so boundary bursts are kept off it.
"""

import sys

if "/opt/trn_rl_repo" not in sys.path:
    sys.path.insert(0, "/opt/trn_rl_repo")

import math

import numpy as np

B, S, D, H = 2, 2048, 4096, 32
HD = D // H          # 128
HLOC = 4             # heads per core
NC = 8               # cores
TOK = B * S          # 4096
CH = TOK // 512      # 8 token chunks of 512 (QK phase)
VCH = 8              # 256-token chunks per batch (V phase)
DKT = D // 128       # 32 contraction tiles
QC = S // 512        # 4 q-chunks per sequence
KT = S // 128        # 16 k-tiles per sequence
ISQRT = 1.0 / math.sqrt(HD)

_CACHE = {}


# --------------------------------------------------------------------------
# causal fast path
# --------------------------------------------------------------------------

def _build_causal(nrep: int = 1):
    import concourse.bacc as bacc
    import concourse.tile as tile
    from concourse import mybir

    F32 = mybir.dt.float32
    F32R = mybir.dt.float32r
    F16 = mybir.dt.float16
    EXP = mybir.ActivationFunctionType.Exp

    nc = bacc.Bacc("TRN2", target_bir_lowering=False, debug=False, num_devices=NC)

    xt16_d = nc.dram_tensor("xt16", [DKT, 128, TOK], F16, kind="ExternalInput")
    wq_d = nc.dram_tensor("wq", [128, DKT, 512], F16, kind="ExternalInput")
    wk_d = nc.dram_tensor("wk", [128, DKT, 512], F16, kind="ExternalInput")
    wv_d = nc.dram_tensor("wv", [128, DKT, 512], F16, kind="ExternalInput")
    wo_d = nc.dram_tensor("wo", [128, HLOC, D], F32R, kind="ExternalInput")
    cs_d = nc.dram_tensor("cs", [128, S], F32, kind="ExternalInput")
    ss_d = nc.dram_tensor("ss", [128, S], F32, kind="ExternalInput")
    mt_d = nc.dram_tensor("mtri", [128, 128], F32, kind="ExternalInput")
    # fp16 partial output: halves the 64MB store, host sums in fp32
    out_d = nc.dram_tensor("out", [TOK, D], F16, kind="ExternalOutput")

    # DRAM scratch for projected Q/K/V, all spilled in fp16: halves the
    # spill+reload traffic and the attention matmuls run fp16 (1 cyc/row at
    # any width, so the narrow diagonal tiles are free)
    qdr = {b: nc.dram_tensor(f"qdr{b}", [HLOC, 128, S], F16) for b in range(B)}
    kdr = {b: nc.dram_tensor(f"kdr{b}", [HLOC, 128, S], F16) for b in range(B)}
    vdr = {b: nc.dram_tensor(f"vdr{b}", [S, 512], F16) for b in range(B)}

    with tile.TileContext(nc) as tc:
        with tc.tile_pool(name="consts", bufs=1) as consts:
            # all-ones [128,128] lhsT: the sums matmul broadcasts the column
            # sums to every partition (same 512-column cost), so no separate
            # partition-broadcast is needed for the normalization
            ones_sb = consts.tile([128, 128], F32R)
            nc.vector.memset(ones_sb.bitcast(F32), 1.0)
            mtri = consts.tile([128, 128], F32, name="mtri")
            # SWDGE queue: keeps the startup HWDGE/scalar path clear for
            # the first weight batches
            nc.gpsimd.dma_start(out=mtri, in_=mt_d.ap())
            for _ in range(nrep):
                _qk_phase(nc, tc, xt16_d, wq_d, wk_d, cs_d, ss_d, qdr, kdr,
                          F32, F32R, F16)
                _vattn_phases(nc, tc, ones_sb, mtri, xt16_d, wv_d,
                              wo_d, cs_d, ss_d, qdr, kdr, vdr, out_d,
                              F32, F32R, F16, EXP)

    nc.compile()
    return nc


def _qk_phase(nc, tc, xt_d, wq_d, wk_d, cs_d, ss_d, qdr, kdr, F32, F32R, F16):
    """Q,K projections emitted in transposed [HD, tok] layout with RoPE.

    Entirely fp16 on the PE (weights + x): same 1 cyc/row as fp32r but half
    the DMA traffic, which un-saturates the bus during chunk 0 (weights + x
    + rope constants used to exceed the 360GB/s budget there).
    """
    with (
        # rope FIRST: its last readers are the final epilogue spill DMAs
        # (~20us after the last matmul), so any W2 pool landing in its
        # address range is gated that long. With rope at the bottom, the
        # W2 x/weight pools land in w1/xt1's range, whose last readers are
        # the final QK matmuls — W2's loads start immediately.
        tc.tile_pool(name="rope", bufs=1) as rope,
        tc.tile_pool(name="w1", bufs=1) as w1,
        tc.tile_pool(name="xt1", bufs=3) as xt1,
        tc.tile_pool(name="ps1", bufs=1, space="PSUM") as ps1,
    ):
        wq_sb = w1.tile([128, DKT, 512], F16, tag="wq")
        wk_sb = w1.tile([128, DKT, 512], F16, tag="wk")

        def load_xt(ch, g):
            xt = xt1.tile([128, 2, 512], F16, name="xt", tag="xt", bufs=5)
            nc.sync.dma_start(
                out=xt,
                in_=xt_d.ap()[2 * g:2 * g + 2, :,
                              ch * 512:(ch + 1) * 512].rearrange(
                    "g p t -> p g t"
                ),
            )
            return xt

        xt_pre = {}
        for ch in range(CH):
            b, s0 = ch // QC, (ch % QC) * 512
            cs_sb = rope.tile([128, 512], F32, name="cs_c", tag="cs_c", bufs=2)
            ss_sb = rope.tile([128, 512], F32, name="ss_c", tag="ss_c", bufs=2)
            qps = [ps1.tile([128, 512], F32, name=f"qps{h}", tag=f"q{h}")
                   for h in range(HLOC)]
            kps = [ps1.tile([128, 512], F32, name=f"kps{h}", tag=f"k{h}")
                   for h in range(HLOC)]
            for g in range(DKT // 2):
                # xt first: the g==0 load is on the critical path to the
                # very first matmul; chunk-boundary tiles were prefetched
                xt = xt_pre.pop(g, None) or load_xt(ch, g)
                if ch == 0:
                    # weights trickle in 2-dk batches one iteration ahead of
                    # use: per-g bus demand stays under the PE's consumption
                    # rate, so neither stream ever starves the other
                    if g == 0:
                        # wk(0:2) on scalar: the Pool queue starts with the
                        # framework's init memsets, which would delay the
                        # first k-matmuls by ~2us
                        nc.scalar.dma_start(
                            out=wq_sb[:, 0:2, :], in_=wq_d.ap()[:, 0:2, :]
                        )
                        nc.scalar.dma_start(
                            out=wk_sb[:, 0:2, :], in_=wk_d.ap()[:, 0:2, :]
                        )
                        nc.scalar.dma_start(
                            out=wq_sb[:, 2:4, :], in_=wq_d.ap()[:, 2:4, :]
                        )
                        nc.gpsimd.dma_start(
                            out=wk_sb[:, 2:4, :], in_=wk_d.ap()[:, 2:4, :]
                        )
                    elif g < 15:
                        lo = 2 * g + 2
                        nc.scalar.dma_start(
                            out=wq_sb[:, lo:lo + 2, :],
                            in_=wq_d.ap()[:, lo:lo + 2, :],
                        )
                        nc.gpsimd.dma_start(
                            out=wk_sb[:, lo:lo + 2, :],
                            in_=wk_d.ap()[:, lo:lo + 2, :],
                        )
                if g == 8:
                    # rope constants mid-loop on the SWDGE queue: off the
                    # startup critical path, well ahead of the epilogue
                    nc.gpsimd.dma_start(out=cs_sb,
                                        in_=cs_d.ap()[:, s0:s0 + 512])
                    nc.gpsimd.dma_start(out=ss_sb,
                                        in_=ss_d.ap()[:, s0:s0 + 512])
                if ch + 1 < CH and g in (13, 14):
                    # prefetch the next chunk's first xt tiles past the
                    # epilogue's DMA burst at the boundary
                    xt_pre[g - 13] = load_xt(ch + 1, g - 13)
                for gg in range(2):
                    dk = 2 * g + gg
                    for h in range(HLOC):
                        nc.tensor.matmul(
                            qps[h], wq_sb[:, dk, h * 128:(h + 1) * 128],
                            xt[:, gg, :],
                            start=(dk == 0), stop=(dk == DKT - 1),
                        )
                    for h in range(HLOC):
                        nc.tensor.matmul(
                            kps[h], wk_sb[:, dk, h * 128:(h + 1) * 128],
                            xt[:, gg, :],
                            start=(dk == 0), stop=(dk == DKT - 1),
                        )
            # epilogue pass 1: drain all 8 PSUM banks first (frees banks for
            # the next chunk) — copies split across ACT and DVE
            pcs = []
            for i, ps in enumerate(qps + kps):
                pc = rope.tile([128, 512], F32, name="pc", tag="pc", bufs=3)
                if i % 2 == 0:
                    nc.scalar.copy(pc, ps)
                else:
                    nc.vector.tensor_copy(pc, ps)
                pcs.append(pc)
            # pass 2: rope products; s-terms in fp16 (halves the swap DMAs),
            # cos-terms kept fp32 so the final value rounds only twice
            tq = rope.tile([128, HLOC, 512], F32, name="tq", tag="tq")
            tk = rope.tile([128, HLOC, 512], F32, name="tk", tag="tk")
            sq = rope.tile([128, HLOC, 512], F16, name="sq", tag="sq")
            sk = rope.tile([128, HLOC, 512], F16, name="sk", tag="sk")
            swq = rope.tile([128, HLOC, 512], F16, name="swq", tag="swq")
            swk = rope.tile([128, HLOC, 512], F16, name="swk", tag="swk")
            for h in range(HLOC):
                nc.vector.tensor_mul(sq[:, h, :], pcs[h], ss_sb)
                nc.vector.tensor_mul(sk[:, h, :], pcs[HLOC + h], ss_sb)
            # partition swaps on the SWDGE queue: keeps the boundary burst
            # off the serialized HWDGE descriptor generator
            nc.gpsimd.dma_start(out=swq[0:64], in_=sq[64:128])
            nc.gpsimd.dma_start(out=swq[64:128], in_=sq[0:64])
            nc.gpsimd.dma_start(out=swk[0:64], in_=sk[64:128])
            nc.gpsimd.dma_start(out=swk[64:128], in_=sk[0:64])
            for h in range(HLOC):
                nc.vector.tensor_mul(tq[:, h, :], pcs[h], cs_sb)
                nc.vector.tensor_mul(tk[:, h, :], pcs[HLOC + h], cs_sb)
            # pass 3: combine into fp16 + batched spill; the fp16 outputs
            # reuse the sq/sk tiles whose last readers are the swap DMAs
            for h in range(HLOC):
                nc.vector.tensor_add(sq[:, h, :], tq[:, h, :], swq[:, h, :])
            nc.scalar.dma_start(
                out=qdr[b].ap()[:, :, s0:s0 + 512].rearrange("h p t -> p h t"),
                in_=sq,
            )
            for h in range(HLOC):
                nc.vector.tensor_add(sk[:, h, :], tk[:, h, :], swk[:, h, :])
            nc.scalar.dma_start(
                out=kdr[b].ap()[:, :, s0:s0 + 512].rearrange("h p t -> p h t"),
                in_=sk,
            )


def _vattn_phases(nc, tc, ones_sb, mtri, xt16_d, wv_d, wo_d, cs_d,
                  ss_d, qdr, kdr, vdr, out_d, F32, F32R, F16, EXP):
    import concourse.bass_isa as bass_isa

    hbs = [(b, h) for b in range(B) for h in range(HLOC)]

    # pools, LIFO-ordered: attention pools first (live to the end), V pools
    # on top (released after W3).
    qkv = tc.alloc_tile_pool(name="qkv", bufs=2)
    hold = tc.alloc_tile_pool(name="hold", bufs=1)
    sm = tc.alloc_tile_pool(name="sm", bufs=2)
    w3p = tc.alloc_tile_pool(name="w3p", bufs=1, side="right")   # wo
    w2 = tc.alloc_tile_pool(name="w2", bufs=1, side="right")     # wv
    # with rope allocated first in W1, xtv/vst land in the freed w1/xt1
    # range: the first xg loads are ungated the moment the last QK matmul
    # retires
    xtv = tc.alloc_tile_pool(name="xtv", bufs=4)
    vst = tc.alloc_tile_pool(name="vst", bufs=2)
    # W2 V(b0) uses 512-token chunks; bufs=2 (8 banks) so the next chunk's
    # matmuls overlap the previous chunk's drains.
    psV0 = tc.alloc_tile_pool(name="psV0", bufs=2, space="PSUM")

    wv_sb = w2.tile([128, DKT - 6, 512], F16, tag="wv")
    # first 6 dk-tiles of wv staged in the xtv pool (freed w1/xt1 range,
    # ungated at the last QK matmul): these transfer during the last QK
    # chunk's epilogue so V(b0) starts immediately
    wvs = xtv.tile([128, 6, 512], F16, tag="wvs", bufs=1)
    for dks in (slice(0, 3), slice(3, 6)):
        nc.sync.dma_start(out=wvs[:, dks, :], in_=wv_d.ap()[:, dks, :])

    wo_sb = w3p.tile([128, HLOC, D], F32R, tag="wo")

    def v_weights(c, g):
        # remaining wv tiles (dk 6-31) on the scalar HWDGE queue (sync is
        # dedicated to JIT xg loads), batched one group ahead of use
        if c != 0 or g % 2 != 1:
            return
        if g == 1:
            nc.scalar.dma_start(out=wv_sb[:, 0:2, :], in_=wv_d.ap()[:, 6:8, :])
        lo = 2 * g + 2
        if 8 <= lo < DKT:
            nc.scalar.dma_start(
                out=wv_sb[:, lo - 6:lo - 2, :], in_=wv_d.ap()[:, lo:lo + 4, :]
            )

    def wv_at(dk):
        return wvs[:, dk, :] if dk < 6 else wv_sb[:, dk - 6, :]

    def v_drain(vps, b, s0, t, eng):
        vc = vst.tile([128, 512], F16, name="vc", tag="vc")
        if eng is nc.scalar:
            nc.scalar.copy(vc, vps)
        else:
            eng.tensor_copy(vc, vps)
        nc.gpsimd.dma_start(
            out=vdr[b].ap()[s0 + t * 128:s0 + (t + 1) * 128, :], in_=vc
        )

    def v_chunk0(c):
        # 512-token chunk of batch 0; xg batched 4-dk to halve the HWDGE
        # descriptor-generation load
        s0 = c * 512
        vps = [psV0.tile([128, 512], F32, name=f"v0ps{t}", tag=f"v0{t}")
               for t in range(4)]
        for gq in range(DKT // 4):
            v_weights(c, 2 * gq)
            v_weights(c, 2 * gq + 1)
            xg = xtv.tile([128, 4, 512], F16, name="xg0", tag="xg")
            nc.sync.dma_start(
                out=xg,
                in_=xt16_d.ap()[4 * gq:4 * gq + 4, :, s0:s0 + 512].rearrange(
                    "g p t -> p g t"
                ),
            )
            for gg in range(4):
                dk = 4 * gq + gg
                for t in range(4):
                    nc.tensor.matmul(
                        vps[t], xg[:, gg, t * 128:(t + 1) * 128],
                        wv_at(dk),
                        start=(dk == 0), stop=(dk == DKT - 1),
                    )
        for t in range(4):
            v_drain(vps[t], 0, s0, t, nc.scalar if t % 2 == 0 else nc.vector)

    xg_pre = []

    def load_xg1(i, g2, tag="xg"):
        s0 = i * 256
        if tag == "xgp":
            xg = xtv.tile([128, 8, 256], F16, name="xgp", tag=tag, bufs=3)
        else:
            xg = xtv.tile([128, 8, 256], F16, name="xg", tag=tag)
        nc.sync.dma_start(
            out=xg,
            in_=xt16_d.ap()[8 * g2:8 * g2 + 8, :,
                            S + s0:S + s0 + 256].rearrange(
                "g p t -> p g t"
            ),
        )
        return xg

    def v_chunk1_half(i, psV, vps, hi):
        # half of a 256-token chunk of batch 1 — emitted in two parts around
        # attention units so the PE has V matmuls to run while exps drain
        s0 = i * 256
        if hi == 0:
            vps[:] = [psV.tile([128, 512], F32, name=f"vps{t}", tag=f"v{t}")
                      for t in range(2)]
        for g2 in range(hi * 2, hi * 2 + 2):
            if i == 0 and xg_pre:
                xg = xg_pre.pop(0)
            else:
                xg = load_xg1(i, g2)
            for gg in range(8):
                dk = 8 * g2 + gg
                for t in range(2):
                    nc.tensor.matmul(
                        vps[t], xg[:, gg, t * 128:(t + 1) * 128],
                        wv_at(dk),
                        start=(dk == 0), stop=(dk == DKT - 1),
                    )
        if hi == 1:
            for t in range(2):
                v_drain(vps[t], 1, s0, t, nc.vector)

    # NOTE: loads must be emitted after the spill writes they read — the tile
    # framework only tracks dependencies on already-emitted instructions.
    # Queue plan: sync = JIT xg loads only; gpsimd carries the wait-free
    # qT/kT reloads (their qdr/kdr spills are long done, so the Pool SEQ
    # never head-of-line blocks); scalar/vector take the rest.
    def load_qk(i, eng_q=None, eng_k=None):
        b, h = hbs[i]
        eng_q = eng_q or nc.gpsimd
        eng_k = eng_k or nc.scalar
        qT = qkv.tile([128, S], F16, name=f"qT{i}", tag="qT")
        kT = qkv.tile([128, S], F16, name=f"kT{i}", tag="kT")
        for half in range(2):
            sl = slice(half * (S // 2), (half + 1) * (S // 2))
            eng_q.dma_start(out=qT[:, sl], in_=qdr[b].ap()[h][:, sl])
            eng_k.dma_start(out=kT[:, sl], in_=kdr[b].ap()[h][:, sl])
        return qT, kT

    def load_v(b, hp, eng):
        # fp16 vT for a PAIR of heads: 512-byte contiguous runs keep the
        # descriptor latency multiplier at 1
        vT = qkv.tile([128, KT, 256], F16, name=f"vT{b}{hp}", tag="vT")
        vsrc = vdr[b].ap()[:, hp * 256:(hp + 1) * 256].rearrange(
            "(n p) d -> p n d", p=128
        )
        for half in range(2):
            sl = slice(half * (KT // 2), (half + 1) * (KT // 2))
            eng.dma_start(out=vT[:, sl, :], in_=vsrc[:, sl, :])
        return vT

    qk_tiles = {}
    v_tiles = {}
    hoTs = {}
    attn_ps = [None, None]   # [hops-pool, st-pool]

    def attn_unit(i, qc):
        ps3, ps4 = attn_ps
        b, h = hbs[i]
        qT, kT = qk_tiles[i]
        vT = v_tiles[(b, h // 2)]
        vc0 = (h % 2) * 128
        hoT = hoTs[b]
        qs = qc * 512
        nkt = (qc + 1) * 4
        hops = ps3.tile([128, 512], F32, name="hops", tag="hops")
        acc = sm.tile([128, 512], F32R, name="acc", tag="acc")
        # software-pipelined three deep: scores(kt+1..kt+3) issue before
        # PV(kt) so the PE never head-of-line waits on the exp latency
        pend = []

        def emit_pv(stop):
            pkt, pw, pex = pend.pop(0)
            nc.tensor.matmul(
                hops[:, pw:], vT[:, pkt, vc0:vc0 + 128], pex[:, pw:],
                start=(pkt == 0), stop=stop,
            )

        for kt in range(nkt):
            j = kt - (nkt - 4)
            # fp16 matmuls run 1 cyc/row at any width, so the triangle
            # narrowing is free (no 4x penalty below 256 cols)
            w = 128 * j if j > 0 else 0
            st = ps4.tile([128, 512], F32, name="st", tag="st")
            nc.tensor.matmul(
                st[:, w:], kT[:, kt * 128:(kt + 1) * 128],
                qT[:, qs + w:qs + 512],
                start=True, stop=True,
            )
            if j >= 0:
                nc.vector.tensor_add(st[:, w:w + 128], st[:, w:w + 128], mtri)
            ex = sm.tile([128, 512], F16, name="ex", tag="ex", bufs=5)
            nc.scalar.activation(ex[:, w:], st[:, w:], EXP, scale=ISQRT)
            if len(pend) == 3:
                emit_pv(stop=False)
            if kt == 0:
                nc.vector.tensor_copy(acc, ex)
            else:
                nc.vector.tensor_add(acc[:, w:], acc[:, w:], ex[:, w:])
            pend.append((kt, w, ex))

        def finish():
            # trailing PVs + normalization, emitted by the caller AFTER the
            # next block of independent matmuls: the PE's in-order queue
            # would otherwise head-of-line stall on the last exps
            while pend:
                emit_pv(stop=(len(pend) == 1))
            # softmax denominator on the idle Pool engine (replaces a
            # 512-col PE ones-matmul per unit); output lands broadcast to
            # every partition
            sums = sm.tile([128, 512], F32, name="sums", tag="sums", bufs=2)
            nc.gpsimd.partition_all_reduce(
                sums, acc.bitcast(F32), 128, bass_isa.ReduceOp.add
            )
            rb = sm.tile([128, 512], F32, name="rb", tag="rb", bufs=1)
            nc.vector.reciprocal(rb, sums)
            nc.vector.tensor_mul(hoT[:, h, qs:qs + 512], hops, rb)

        return finish

    # ---- W2: V projection batch 0 ----
    for c in range(4):
        v_chunk0(c)
        if c == 1:
            # first attention loads mid-W2: past the W1->W2 boundary's
            # HWDGE burst, well ahead of W3's first units
            qk_tiles[0] = load_qk(0)
        if c == 2:
            # prefetch the whole first W3 V-chunk's x past the W2->W3
            # boundary
            for g2 in range(4):
                xg_pre.append(load_xg1(0, g2, tag="xgp"))
    # vT reads every token row of vdr[0], so this load may only be emitted
    # once ALL four chunks' drains are emitted (the tile framework orders
    # loads only against already-emitted writes)
    v_tiles[(0, 0)] = load_v(0, 0, nc.scalar)
    psV0.release()

    # ---- W3: V projection batch 1 interleaved with attention(b0) ----
    ps3 = tc.alloc_tile_pool(name="ps3", bufs=2, space="PSUM")   # hops
    ps4 = tc.alloc_tile_pool(name="ps4", bufs=4, space="PSUM")   # st
    psV = tc.alloc_tile_pool(name="psV", bufs=1, space="PSUM")
    attn_ps[:] = [ps3, ps4]
    hoTs[0] = hold.tile([128, HLOC, S], F32R, name="hoT0", tag="hoT", bufs=1)
    fin_prev = None
    for i in range(VCH):
        vps = []
        for u in (2 * i, 2 * i + 1):
            v_chunk1_half(i, psV, vps, u % 2)
            if fin_prev is not None:
                # previous unit's trailing PVs, now covered by the V-half
                # matmuls just emitted
                fin_prev()
            h, qc = divmod(u, QC)
            if qc == 0 and h + 1 < HLOC and h + 1 not in qk_tiles:
                qk_tiles[h + 1] = load_qk(h + 1)
                if h == 1:
                    v_tiles[(0, 1)] = load_v(0, 1, nc.scalar)
            if u == 5:
                # b1 h0 q/k only depend on W1 spills: load early, spread out
                qk_tiles[HLOC] = load_qk(HLOC)
            if u in (2, 6, 10, 13):
                # wo loads spread across W3 so W4's first O-pieces (which
                # touch every wo column) never wait on them
                ho = {2: 0, 6: 1, 10: 2, 13: 3}[u]
                nc.scalar.dma_start(out=wo_sb[:, ho, :],
                                    in_=wo_d.ap()[:, ho, :])
            fin_prev = attn_unit(h, qc)
            if u == 15:
                # (b1,h0) V: only now are ALL vdr[1] spill writes emitted
                # (this half contained the last chunk's drain); emitted after
                # the unit so the brief SEQ wait cannot stall its exps
                v_tiles[(1, 0)] = load_v(1, 0, nc.scalar)
    fin_prev()
    fin_prev = None

    psV.release()
    vst.release()
    xtv.release()
    w2.release()

    # ---- W4: attention(b1) interleaved with O-projection(b0) ----
    ost = tc.alloc_tile_pool(name="ost", bufs=4)
    psO = tc.alloc_tile_pool(name="psO", bufs=2, space="PSUM")

    def o_piece(b, qc, t, half, hoT, three_way, split_store=False):
        # one (t, half) piece: 4 output-column chains + two stores
        c0 = qc * 512 + t * 128
        for pair in range(2):
            ot = ost.tile([128, 2, 512], F16, name="ot", tag="ot")
            for oi in range(2):
                oc = half * 4 + pair * 2 + oi
                if three_way and (pair + oi) % 2 == 1:
                    ops = attn_ps[1].tile([128, 512], F32, name="ops2",
                                          tag="st")
                else:
                    ops = psO.tile([128, 512], F32, name="ops", tag="ops")
                for h in range(HLOC):
                    nc.tensor.matmul(
                        ops, hoT[:, h, c0:c0 + 128],
                        wo_sb[:, h, oc * 512:(oc + 1) * 512],
                        start=(h == 0), stop=(h == HLOC - 1),
                    )
                # drains alternate ACT/DVE (gpsimd cannot access PSUM)
                if (pair * 2 + oi) % 2 == 0:
                    nc.scalar.copy(ot[:, oi, :], ops)
                else:
                    nc.vector.tensor_copy(ot[:, oi, :], ops)
                if split_store:
                    # kernel tail: per-column stores on alternating queues
                    # so the final store after the last drain is small
                    (nc.scalar if oi == 0 else nc.sync).dma_start(
                        out=out_d.ap()[b * S + c0:b * S + c0 + 128,
                                       oc * 512:(oc + 1) * 512],
                        in_=ot[:, oi, :],
                    )
            if not split_store:
                (nc.scalar if (pair + half) % 2 == 0 else nc.sync).dma_start(
                    out=out_d.ap()[b * S + c0:b * S + c0 + 128,
                                   (half * 4 + pair * 2) * 512:
                                   (half * 4 + pair * 2 + 2) * 512],
                    in_=ot,
                )

    def o_block(b, qc, hoT, three_way=False, last=False):
        for t in range(4):
            for half in range(2):
                o_piece(b, qc, t, half, hoT, three_way,
                        split_store=(last and t == 3 and half == 1))

    hold1 = tc.alloc_tile_pool(name="hold1", bufs=1, side="right")
    hoTs[1] = hold1.tile([128, HLOC, S], F32R, name="hoT1", tag="hoT1")
    # O(b0) is entirely ready at W4 start: two of its 32 pieces follow every
    # attention(b1) unit, filling the exp-latency PE slack uniformly
    opieces = [(qc, t, half)
               for qc in range(QC) for t in range(4) for half in range(2)]
    for u in range(HLOC * QC):
        h, qc = divmod(u, QC)
        i = HLOC + h
        if qc == 0 and h + 1 < HLOC and (i + 1) not in qk_tiles:
            qk_tiles[i + 1] = load_qk(i + 1)
            if h == 1:
                v_tiles[(1, 1)] = load_v(1, 1, nc.scalar)
        fin = attn_unit(i, qc)
        for p in (2 * u, 2 * u + 1):
            pqc, pt, phalf = opieces[p]
            o_piece(0, pqc, pt, phalf, hoTs[0], False)
        fin()

    # ---- W5: O-projection(b1) ----
    for qc in range(QC):
        o_block(1, qc, hoTs[1], three_way=True, last=(qc == QC - 1))

    psO.release()
    ost.release()
    hold1.release()
    for p in reversed(attn_ps):
        p.release()
    sm.release()
    hold.release()
    w3p.release()
    qkv.release()


# --------------------------------------------------------------------------
# legacy generic path (non-causal masks)
# --------------------------------------------------------------------------

def _build_legacy():
    import concourse.bacc as bacc
    import concourse.tile as tile
    from concourse import mybir

    F32 = mybir.dt.float32
    F32R = mybir.dt.float32r
    EXP = mybir.ActivationFunctionType.Exp

    nc = bacc.Bacc("TRN2", target_bir_lowering=False, debug=False, num_devices=NC)

    xt_d = nc.dram_tensor("xt", [DKT, 128, TOK], F32R, kind="ExternalInput")
    wq_d = nc.dram_tensor("wq", [128, DKT, 512], F32R, kind="ExternalInput")
    wk_d = nc.dram_tensor("wk", [128, DKT, 512], F32R, kind="ExternalInput")
    wv_d = nc.dram_tensor("wv", [128, DKT, 512], F32R, kind="ExternalInput")
    wo_d = nc.dram_tensor("wo", [128, HLOC, D], F32R, kind="ExternalInput")
    cs_d = nc.dram_tensor("cs", [128, S], F32, kind="ExternalInput")
    ss_d = nc.dram_tensor("ss", [128, S], F32, kind="ExternalInput")
    mk_d = nc.dram_tensor("maskf", [KT, 128, S], F32, kind="ExternalInput")
    out_d = nc.dram_tensor("out", [TOK, D], F32, kind="ExternalOutput")

    qdr = {(h, b): nc.dram_tensor(f"qdr{h}_{b}", [128, S], F32R)
           for h in range(HLOC) for b in range(B)}
    kdr = {(h, b): nc.dram_tensor(f"kdr{h}_{b}", [128, S], F32R)
           for h in range(HLOC) for b in range(B)}
    vdr = {b: nc.dram_tensor(f"vdr{b}", [S, 512], F32R) for b in range(B)}

    with tile.TileContext(nc) as tc:
        with tc.tile_pool(name="consts", bufs=1) as consts:
            ones_sb = consts.tile([128, 1], F32R)
            nc.vector.memset(ones_sb.bitcast(F32), 1.0)

            # Phase 1b: V projection
            with (
                tc.tile_pool(name="w2", bufs=1) as w2,
                tc.tile_pool(name="xt2", bufs=4) as xt2,
                tc.tile_pool(name="vcp", bufs=4) as vcp,
                tc.tile_pool(name="ps2", bufs=2, space="PSUM") as ps2,
            ):
                wv_sb = w2.tile([128, DKT, 512], F32R, tag="wv")
                for ch in range(CH):
                    b, s0 = ch // QC, (ch % QC) * 512
                    vps = [ps2.tile([128, 512], F32, name=f"vps{t}", tag=f"v{t}")
                           for t in range(4)]
                    for dk in range(DKT):
                        if ch == 0:
                            we = nc.scalar if dk % 2 == 0 else nc.sync
                            we.dma_start(out=wv_sb[:, dk, :], in_=wv_d.ap()[:, dk, :])
                        xt = xt2.tile([128, 512], F32R, name="xt", tag="xt")
                        nc.sync.dma_start(
                            out=xt, in_=xt_d.ap()[dk, :, ch * 512:(ch + 1) * 512]
                        )
                        for t in range(4):
                            nc.tensor.matmul(
                                vps[t], xt[:, t * 128:(t + 1) * 128], wv_sb[:, dk, :],
                                start=(dk == 0), stop=(dk == DKT - 1),
                            )
                    for t in range(4):
                        vc = vcp.tile([128, 512], F32R, tag="vc")
                        nc.vector.tensor_copy(vc, vps[t])
                        nc.gpsimd.dma_start(
                            out=vdr[b].ap()[s0 + t * 128:s0 + (t + 1) * 128, :],
                            in_=vc,
                        )

            # Phase 1a: Q,K projections + RoPE
            with (
                tc.tile_pool(name="w1", bufs=1) as w1,
                tc.tile_pool(name="xt1", bufs=4) as xt1,
                tc.tile_pool(name="rope", bufs=2) as rope,
                tc.tile_pool(name="ps1", bufs=1, space="PSUM") as ps1,
            ):
                wq_sb = w1.tile([128, DKT, 512], F32R, tag="wq")
                wk_sb = w1.tile([128, DKT, 512], F32R, tag="wk")
                for ch in range(CH):
                    b, s0 = ch // QC, (ch % QC) * 512
                    cs_sb = rope.tile([128, 512], F32, name="cs_c", tag="cs_c")
                    ss_sb = rope.tile([128, 512], F32, name="ss_c", tag="ss_c")
                    nc.scalar.dma_start(out=cs_sb, in_=cs_d.ap()[:, s0:s0 + 512])
                    nc.scalar.dma_start(out=ss_sb, in_=ss_d.ap()[:, s0:s0 + 512])
                    qps = [ps1.tile([128, 512], F32, name=f"qps{h}", tag=f"q{h}")
                           for h in range(HLOC)]
                    kps = [ps1.tile([128, 512], F32, name=f"kps{h}", tag=f"k{h}")
                           for h in range(HLOC)]
                    for dk in range(DKT):
                        if ch == 0:
                            we = nc.scalar if dk % 2 == 0 else nc.sync
                            wf = nc.sync if dk % 2 == 0 else nc.scalar
                            we.dma_start(out=wq_sb[:, dk, :], in_=wq_d.ap()[:, dk, :])
                            wf.dma_start(out=wk_sb[:, dk, :], in_=wk_d.ap()[:, dk, :])
                        xt = xt1.tile([128, 512], F32R, name="xt", tag="xt")
                        nc.sync.dma_start(
                            out=xt, in_=xt_d.ap()[dk, :, ch * 512:(ch + 1) * 512]
                        )
                        for h in range(HLOC):
                            nc.tensor.matmul(
                                qps[h], wq_sb[:, dk, h * 128:(h + 1) * 128], xt,
                                start=(dk == 0), stop=(dk == DKT - 1),
                            )
                        for h in range(HLOC):
                            nc.tensor.matmul(
                                kps[h], wk_sb[:, dk, h * 128:(h + 1) * 128], xt,
                                start=(dk == 0), stop=(dk == DKT - 1),
                            )
                    work = []
                    for h in range(HLOC):
                        for ps, dst in ((qps[h], qdr), (kps[h], kdr)):
                            pc = rope.tile([128, 512], F32, name="pc", tag="pc",
                                           bufs=4)
                            t1 = rope.tile([128, 512], F32, name="t1", tag="t1",
                                           bufs=8)
                            s1 = rope.tile([128, 512], F32, name="s1", tag="s1",
                                           bufs=2)
                            s1w = rope.tile([128, 512], F32, name="s1w", tag="s1w",
                                            bufs=8)
                            nc.vector.tensor_copy(pc, ps)
                            nc.vector.tensor_mul(t1, pc, cs_sb)
                            nc.vector.tensor_mul(s1, pc, ss_sb)
                            nc.scalar.dma_start(out=s1w[0:64, :], in_=s1[64:128, :])
                            nc.scalar.dma_start(out=s1w[64:128, :], in_=s1[0:64, :])
                            work.append((h, dst, t1, s1w))
                    for h, dst, t1, s1w in work:
                        rr = rope.tile([128, 512], F32R, name="rr", tag="rr", bufs=2)
                        nc.vector.tensor_add(rr, t1, s1w)
                        nc.scalar.dma_start(out=dst[(h, b)].ap()[:, s0:s0 + 512],
                                            in_=rr)

            # Phases 2+3
            hbs = [(b, h) for b in range(B) for h in range(HLOC)]
            with (
                tc.tile_pool(name="qkv", bufs=2) as qkv,
                tc.tile_pool(name="hold", bufs=1) as hold,
                tc.tile_pool(name="smp", bufs=2) as smp,
                tc.tile_pool(name="ps3", bufs=1, space="PSUM") as ps3,
                tc.tile_pool(name="ps4", bufs=3, space="PSUM") as ps4,
            ):
                def load_hb(i):
                    b, h = hbs[i]
                    qT = qkv.tile([128, S], F32R, name=f"qT_{i}", tag="qT")
                    kT = qkv.tile([128, S], F32R, name=f"kT_{i}", tag="kT")
                    vT = qkv.tile([128, KT, 128], F32R, name=f"vT_{i}", tag="vT")
                    vsrc = vdr[b].ap()[:, h * 128:(h + 1) * 128].rearrange(
                        "(n p) d -> p n d", p=128
                    )
                    for j in range(QC):
                        sl = slice(j * 512, (j + 1) * 512)
                        nc.sync.dma_start(out=qT[:, sl], in_=qdr[(h, b)].ap()[:, sl])
                        nc.sync.dma_start(out=kT[:, sl], in_=kdr[(h, b)].ap()[:, sl])
                        nc.sync.dma_start(
                            out=vT[:, j * 4:(j + 1) * 4, :],
                            in_=vsrc[:, j * 4:(j + 1) * 4, :],
                        )
                    return qT, kT, vT

                tiles = {0: load_hb(0)}
                wo_sb = hold.tile([128, HLOC, D], F32R, tag="wo")
                for h in range(HLOC):
                    nc.scalar.dma_start(out=wo_sb[:, h, :], in_=wo_d.ap()[:, h, :])

                hoTs = {}
                for i, (b, h) in enumerate(hbs):
                    if h == 0:
                        hoTs[b] = hold.tile([128, HLOC, S], F32R,
                                            name=f"hoT_{b}", tag=f"hoT{b}")
                    hoT = hoTs[b]
                    if i + 1 < len(hbs):
                        tiles[i + 1] = load_hb(i + 1)
                    qT, kT, vT = tiles.pop(i)
                    for qc in range(QC):
                        qs = qc * 512
                        sums = ps3.tile([1, 512], F32, name="sums", tag="sums")
                        hops = ps3.tile([128, 512], F32, name="hops", tag="hops")
                        for kt in range(KT):
                            st = ps4.tile([128, 512], F32, name="st", tag="st")
                            nc.tensor.matmul(
                                st, kT[:, kt * 128:(kt + 1) * 128],
                                qT[:, qs:qs + 512], start=True, stop=True,
                            )
                            mkt = smp.tile([128, 512], F32, name="mkt", tag="mkt")
                            nc.sync.dma_start(out=mkt, in_=mk_d.ap()[kt, :, qs:qs + 512])
                            nc.vector.tensor_add(st, st, mkt)
                            ex = smp.tile([128, 512], F32R, name="ex", tag="ex",
                                          bufs=4)
                            nc.scalar.activation(ex, st, EXP, scale=ISQRT)
                            nc.tensor.matmul(sums, ones_sb, ex, start=(kt == 0),
                                             stop=(kt == KT - 1))
                            nc.tensor.matmul(hops, vT[:, kt, :], ex, start=(kt == 0),
                                             stop=(kt == KT - 1))
                        recip = smp.tile([1, 512], F32, name="recip", tag="recip")
                        nc.vector.reciprocal(recip, sums)
                        bc = smp.tile([128, 512], F32, name="bc", tag="bc")
                        nc.gpsimd.partition_broadcast(bc, recip)
                        nc.vector.tensor_mul(hoT[:, h, qs:qs + 512], hops, bc)

                for b in range(B):
                    with (
                        tc.tile_pool(name=f"oc{b}", bufs=3) as ocp,
                        tc.tile_pool(name=f"ps5{b}", bufs=3, space="PSUM") as ps5,
                    ):
                        for t in range(S // 128):
                            for oc in range(D // 512):
                                ops = ps5.tile([128, 512], F32, name="ops", tag="ops")
                                for h in range(HLOC):
                                    nc.tensor.matmul(
                                        ops, hoTs[b][:, h, t * 128:(t + 1) * 128],
                                        wo_sb[:, h, oc * 512:(oc + 1) * 512],
                                        start=(h == 0), stop=(h == HLOC - 1),
                                    )
                                ot = ocp.tile([128, 512], F32, name="ot", tag="ot")
                                nc.vector.tensor_copy(ot, ops)
                                nc.scalar.dma_start(
                                    out=out_d.ap()[
                                        b * S + t * 128:b * S + (t + 1) * 128,
                                        oc * 512:(oc + 1) * 512,
                                    ],
                                    in_=ot,
                                )

    nc.compile()
    return nc


def _get_nc(causal: bool):
    if causal not in _CACHE:
        _CACHE[causal] = _build_causal() if causal else _build_legacy()
    return _CACHE[causal]


def _host_prep(x, wq, wk, wv, wo, freqs_cos, freqs_sin, mask):
    """Build per-core input maps."""
    x2 = np.ascontiguousarray(x.reshape(TOK, D).T)          # [D, TOK]
    xt = x2.reshape(DKT, 128, TOK)

    cs = np.concatenate([freqs_cos.T, freqs_cos.T], axis=0).astype(np.float32)
    ss = np.concatenate([freqs_sin.T, -freqs_sin.T], axis=0).astype(np.float32)

    m2 = np.asarray(mask, dtype=np.float32).reshape(S, S)
    tril = np.tril(np.ones((S, S), dtype=bool))
    causal = bool(np.all(m2[tril] == 0.0) and np.all(m2[~tril] <= -1e8))
    if causal:
        mk = np.ascontiguousarray(m2[:128, :128].T)         # [k,q] triangle
    else:
        mk = np.ascontiguousarray(m2.T.reshape(KT, 128, S))

    # per-head column permutation: evens then odds (RoPE rotate-half form)
    perm = np.concatenate([np.arange(0, HD, 2), np.arange(1, HD, 2)])

    xt16 = xt.astype(np.float16)

    in_maps = []
    for c in range(NC):
        cols = np.concatenate(
            [(4 * c + h) * HD + perm for h in range(HLOC)]
        )
        wq_c = np.ascontiguousarray(
            wq[:, cols].reshape(DKT, 128, 512).transpose(1, 0, 2)
        )
        wk_c = np.ascontiguousarray(
            wk[:, cols].reshape(DKT, 128, 512).transpose(1, 0, 2)
        )
        vcols = np.arange(4 * c * HD, 4 * (c + 1) * HD)
        wv_c = np.ascontiguousarray(
            wv[:, vcols].reshape(DKT, 128, 512).transpose(1, 0, 2)
        )
        wo_c = np.ascontiguousarray(
            wo[vcols, :].reshape(HLOC, 128, D).transpose(1, 0, 2)
        )
        if causal:
            wq_c = wq_c.astype(np.float16)
            wk_c = wk_c.astype(np.float16)
            wv_c = wv_c.astype(np.float16)
            m = {
                "xt16": xt16, "wq": wq_c, "wk": wk_c, "wv": wv_c,
                "wo": wo_c, "cs": cs, "ss": ss, "mtri": mk,
            }
        else:
            m = {
                "xt": xt, "wq": wq_c, "wk": wk_c, "wv": wv_c, "wo": wo_c,
                "cs": cs, "ss": ss, "maskf": mk,
            }
        in_maps.append(m)
    return in_maps, causal


def kernel(x, wq, wk, wv, wo, freqs_cos, freqs_sin, mask, **_unused):
    from concourse.bass_utils import run_bass_kernel_spmd

    x = np.asarray(x, dtype=np.float32)
    wq = np.asarray(wq, dtype=np.float32)
    wk = np.asarray(wk, dtype=np.float32)
    wv = np.asarray(wv, dtype=np.float32)
    wo = np.asarray(wo, dtype=np.float32)
    freqs_cos = np.asarray(freqs_cos, dtype=np.float32)
    freqs_sin = np.asarray(freqs_sin, dtype=np.float32)

    in_maps, causal = _host_prep(x, wq, wk, wv, wo, freqs_cos, freqs_sin, mask)
    nc = _get_nc(causal)
    res = run_bass_kernel_spmd(nc, in_maps, list(range(NC)))
    out = res.results[0]["out"].astype(np.float32)
    for c in range(1, NC):
        out = out + res.results[c]["out"].astype(np.float32)
    return out.reshape(B, S, D).astype(np.float32)



# revision 73
# speedup vs baseline: 1.0006x; 1.0006x over previous
"""Trainium2 Bass kernel for multi-head causal attention with RoPE.

Model (per reference):
  B=2, S=2048, D=4096, H=32 heads, HD=128.
  out = softmax(rope(x@wq) @ rope(x@wk)^T / sqrt(HD) + mask) @ (x@wv) @ wo

Sharding: tensor-parallel over heads. Core c in 0..7 owns heads 4c..4c+3:
wq/wk/wv column-sharded, wo row-sharded; each core produces a full-shape
partial output and the host sums the 8 partials (the all-reduce).

Precision: all projections and attention matmuls run fp16 (same 1 cyc/row
as fp32r on the PE but half the DMA traffic, and no 4x penalty on
sub-256-col tiles, so the triangle-narrowed diagonals are free). PSUM
accumulation stays fp32; rope combines in fp32 and rounds once to fp16 at
the spill; the O-projection (hoT @ wo) stays fp32r. Measured rel err
1.72e-2 vs the 2e-2 gate (deterministic: fixed seed).

Causal fast path — 5 overlapped windows keeping the PE stall-free:
  W1  QK projections (512-token chunks, 8 PSUM banks), RoPE epilogue with
      PSUM drains split across ACT+DVE, fp16 partition swaps on the SWDGE
      queue, batched fp16 spills. Weights trickle in 2-dk batches one
      iteration ahead; xt tiles prefetch across chunk boundaries. The rope
      pool is allocated FIRST so the W2 pools land in w1/xt1's address
      range, whose last readers are the final matmuls — not the epilogue
      spills — ungating W2's loads ~20us earlier.
  W2  V projection batch 0 (512-token chunks, 2x4 PSUM banks); first
      attention loads and W3 x prefetches mid-window.
  W3  V projection batch 1 interleaved with attention(b0); wo loads
      spread across the window on the scalar queue.
  W4  attention(b1) interleaved with O-projection(b0), two o_pieces after
      every unit.
  W5  O-projection(b1), three-way PSUM rotation, split tail stores.
Attention per (h,qc) unit: fp16 scores with triangle-narrowed diagonal
tiles, exp on ACT to fp16, PV accumulated in PSUM, software-pipelined
three deep (scores kt+1..kt+3 issue before PV kt) with the trailing PVs
emitted by the caller AFTER the next block of independent matmuls (the
PE's in-order queue would otherwise head-of-line stall on the last exps).
Softmax denominator: fp32 DVE accumulation of ex, reduced across
partitions by gpsimd partition_all_reduce (replaces a 512-col PE
ones-matmul per unit), reciprocal+mul on DVE.
Queue plan: sync = JIT x loads only; scalar = weights/spills/attention
loads; SWDGE (gpsimd) = wait-free loads (wk, rope consts, qT reloads,
swaps) — a waiting SWDGE/HWDGE dma head-of-line blocks its engine's whole
sequencer, and HWDGE descriptor generation (~0.63us/DMA) is serialized
across the scalar+sync queues, so boundary bursts are kept off it.
"""

import sys

if "/opt/trn_rl_repo" not in sys.path:
    sys.path.insert(0, "/opt/trn_rl_repo")

import math

import numpy as np

B, S, D, H = 2, 2048, 4096, 32
HD = D // H          # 128
HLOC = 4             # heads per core
NC = 8               # cores
TOK = B * S          # 4096
CH = TOK // 512      # 8 token chunks of 512 (QK phase)
VCH = 8              # 256-token chunks per batch (V phase)
DKT = D // 128       # 32 contraction tiles
QC = S // 512        # 4 q-chunks per sequence
KT = S // 128        # 16 k-tiles per sequence
ISQRT = 1.0 / math.sqrt(HD)

_CACHE = {}


# --------------------------------------------------------------------------
# causal fast path
# --------------------------------------------------------------------------

def _build_causal(nrep: int = 1):
    import concourse.bacc as bacc
    import concourse.tile as tile
    from concourse import mybir

    F32 = mybir.dt.float32
    F32R = mybir.dt.float32r
    F16 = mybir.dt.float16
    EXP = mybir.ActivationFunctionType.Exp

    nc = bacc.Bacc("TRN2", target_bir_lowering=False, debug=False, num_devices=NC)

    xt16_d = nc.dram_tensor("xt16", [DKT, 128, TOK], F16, kind="ExternalInput")
    wq_d = nc.dram_tensor("wq", [128, DKT, 512], F16, kind="ExternalInput")
    wk_d = nc.dram_tensor("wk", [128, DKT, 512], F16, kind="ExternalInput")
    wv_d = nc.dram_tensor("wv", [128, DKT, 512], F16, kind="ExternalInput")
    wo_d = nc.dram_tensor("wo", [128, HLOC, D], F32R, kind="ExternalInput")
    cs_d = nc.dram_tensor("cs", [128, S], F32, kind="ExternalInput")
    ss_d = nc.dram_tensor("ss", [128, S], F32, kind="ExternalInput")
    mt_d = nc.dram_tensor("mtri", [128, 128], F32, kind="ExternalInput")
    # fp16 partial output: halves the 64MB store, host sums in fp32
    out_d = nc.dram_tensor("out", [TOK, D], F16, kind="ExternalOutput")

    # DRAM scratch for projected Q/K/V, all spilled in fp16: halves the
    # spill+reload traffic and the attention matmuls run fp16 (1 cyc/row at
    # any width, so the narrow diagonal tiles are free)
    qdr = {b: nc.dram_tensor(f"qdr{b}", [HLOC, 128, S], F16) for b in range(B)}
    kdr = {b: nc.dram_tensor(f"kdr{b}", [HLOC, 128, S], F16) for b in range(B)}
    vdr = {b: nc.dram_tensor(f"vdr{b}", [S, 512], F16) for b in range(B)}

    with tile.TileContext(nc) as tc:
        with tc.tile_pool(name="consts", bufs=1) as consts:
            # all-ones [128,128] lhsT: the sums matmul broadcasts the column
            # sums to every partition (same 512-column cost), so no separate
            # partition-broadcast is needed for the normalization
            ones_sb = consts.tile([128, 128], F32R)
            nc.vector.memset(ones_sb.bitcast(F32), 1.0)
            mtri = consts.tile([128, 128], F32, name="mtri")
            # SWDGE queue: keeps the startup HWDGE/scalar path clear for
            # the first weight batches
            nc.gpsimd.dma_start(out=mtri, in_=mt_d.ap())
            for _ in range(nrep):
                _qk_phase(nc, tc, xt16_d, wq_d, wk_d, cs_d, ss_d, qdr, kdr,
                          F32, F32R, F16)
                _vattn_phases(nc, tc, ones_sb, mtri, xt16_d, wv_d,
                              wo_d, cs_d, ss_d, qdr, kdr, vdr, out_d,
                              F32, F32R, F16, EXP)

    nc.compile()
    return nc


def _qk_phase(nc, tc, xt_d, wq_d, wk_d, cs_d, ss_d, qdr, kdr, F32, F32R, F16):
    """Q,K projections emitted in transposed [HD, tok] layout with RoPE.

    Entirely fp16 on the PE (weights + x): same 1 cyc/row as fp32r but half
    the DMA traffic, which un-saturates the bus during chunk 0 (weights + x
    + rope constants used to exceed the 360GB/s budget there).
    """
    with (
        # rope FIRST: its last readers are the final epilogue spill DMAs
        # (~20us after the last matmul), so any W2 pool landing in its
        # address range is gated that long. With rope at the bottom, the
        # W2 x/weight pools land in w1/xt1's range, whose last readers are
        # the final QK matmuls — W2's loads start immediately.
        tc.tile_pool(name="rope", bufs=1) as rope,
        tc.tile_pool(name="w1", bufs=1) as w1,
        tc.tile_pool(name="xt1", bufs=3) as xt1,
        tc.tile_pool(name="ps1", bufs=1, space="PSUM") as ps1,
    ):
        wq_sb = w1.tile([128, DKT, 512], F16, tag="wq")
        wk_sb = w1.tile([128, DKT, 512], F16, tag="wk")

        def load_xt(ch, g):
            xt = xt1.tile([128, 2, 512], F16, name="xt", tag="xt", bufs=6)
            nc.sync.dma_start(
                out=xt,
                in_=xt_d.ap()[2 * g:2 * g + 2, :,
                              ch * 512:(ch + 1) * 512].rearrange(
                    "g p t -> p g t"
                ),
            )
            return xt

        xt_pre = {}
        for ch in range(CH):
            b, s0 = ch // QC, (ch % QC) * 512
            cs_sb = rope.tile([128, 512], F32, name="cs_c", tag="cs_c", bufs=2)
            ss_sb = rope.tile([128, 512], F32, name="ss_c", tag="ss_c", bufs=2)
            qps = [ps1.tile([128, 512], F32, name=f"qps{h}", tag=f"q{h}")
                   for h in range(HLOC)]
            kps = [ps1.tile([128, 512], F32, name=f"kps{h}", tag=f"k{h}")
                   for h in range(HLOC)]
            for g in range(DKT // 2):
                # xt first: the g==0 load is on the critical path to the
                # very first matmul; chunk-boundary tiles were prefetched
                xt = xt_pre.pop(g, None) or load_xt(ch, g)
                if ch == 0:
                    # weights trickle in 2-dk batches one iteration ahead of
                    # use: per-g bus demand stays under the PE's consumption
                    # rate, so neither stream ever starves the other
                    if g == 0:
                        # wk(0:2) on scalar: the Pool queue starts with the
                        # framework's init memsets, which would delay the
                        # first k-matmuls by ~2us
                        nc.scalar.dma_start(
                            out=wq_sb[:, 0:2, :], in_=wq_d.ap()[:, 0:2, :]
                        )
                        nc.scalar.dma_start(
                            out=wk_sb[:, 0:2, :], in_=wk_d.ap()[:, 0:2, :]
                        )
                        nc.scalar.dma_start(
                            out=wq_sb[:, 2:4, :], in_=wq_d.ap()[:, 2:4, :]
                        )
                        nc.gpsimd.dma_start(
                            out=wk_sb[:, 2:4, :], in_=wk_d.ap()[:, 2:4, :]
                        )
                    elif g < 15:
                        lo = 2 * g + 2
                        nc.scalar.dma_start(
                            out=wq_sb[:, lo:lo + 2, :],
                            in_=wq_d.ap()[:, lo:lo + 2, :],
                        )
                        nc.gpsimd.dma_start(
                            out=wk_sb[:, lo:lo + 2, :],
                            in_=wk_d.ap()[:, lo:lo + 2, :],
                        )
                if g == 8:
                    # rope constants mid-loop on the SWDGE queue: off the
                    # startup critical path, well ahead of the epilogue
                    nc.gpsimd.dma_start(out=cs_sb,
                                        in_=cs_d.ap()[:, s0:s0 + 512])
                    nc.gpsimd.dma_start(out=ss_sb,
                                        in_=ss_d.ap()[:, s0:s0 + 512])
                if ch + 1 < CH and g in (13, 14):
                    # prefetch the next chunk's first xt tiles past the
                    # epilogue's DMA burst at the boundary
                    xt_pre[g - 13] = load_xt(ch + 1, g - 13)
                for gg in range(2):
                    dk = 2 * g + gg
                    for h in range(HLOC):
                        nc.tensor.matmul(
                            qps[h], wq_sb[:, dk, h * 128:(h + 1) * 128],
                            xt[:, gg, :],
                            start=(dk == 0), stop=(dk == DKT - 1),
                        )
                    for h in range(HLOC):
                        nc.tensor.matmul(
                            kps[h], wk_sb[:, dk, h * 128:(h + 1) * 128],
                            xt[:, gg, :],
                            start=(dk == 0), stop=(dk == DKT - 1),
                        )
            # epilogue pass 1: drain all 8 PSUM banks first (frees banks for
            # the next chunk) — copies split across ACT and DVE
            pcs = []
            for i, ps in enumerate(qps + kps):
                pc = rope.tile([128, 512], F32, name="pc", tag="pc", bufs=4)
                if i % 2 == 0:
                    nc.scalar.copy(pc, ps)
                else:
                    nc.vector.tensor_copy(pc, ps)
                pcs.append(pc)
            # pass 2: rope products; s-terms in fp16 (halves the swap DMAs),
            # cos-terms kept fp32 so the final value rounds only twice
            tq = rope.tile([128, HLOC, 512], F32, name="tq", tag="tq")
            tk = rope.tile([128, HLOC, 512], F32, name="tk", tag="tk")
            sq = rope.tile([128, HLOC, 512], F16, name="sq", tag="sq")
            sk = rope.tile([128, HLOC, 512], F16, name="sk", tag="sk")
            swq = rope.tile([128, HLOC, 512], F16, name="swq", tag="swq")
            swk = rope.tile([128, HLOC, 512], F16, name="swk", tag="swk")
            for h in range(HLOC):
                nc.vector.tensor_mul(sq[:, h, :], pcs[h], ss_sb)
                nc.vector.tensor_mul(sk[:, h, :], pcs[HLOC + h], ss_sb)
            # partition swaps on the SWDGE queue: keeps the boundary burst
            # off the serialized HWDGE descriptor generator
            nc.gpsimd.dma_start(out=swq[0:64], in_=sq[64:128])
            nc.gpsimd.dma_start(out=swq[64:128], in_=sq[0:64])
            nc.gpsimd.dma_start(out=swk[0:64], in_=sk[64:128])
            nc.gpsimd.dma_start(out=swk[64:128], in_=sk[0:64])
            for h in range(HLOC):
                nc.vector.tensor_mul(tq[:, h, :], pcs[h], cs_sb)
                nc.vector.tensor_mul(tk[:, h, :], pcs[HLOC + h], cs_sb)
            # pass 3: combine into fp16 + batched spill; the fp16 outputs
            # reuse the sq/sk tiles whose last readers are the swap DMAs
            for h in range(HLOC):
                nc.vector.tensor_add(sq[:, h, :], tq[:, h, :], swq[:, h, :])
            nc.scalar.dma_start(
                out=qdr[b].ap()[:, :, s0:s0 + 512].rearrange("h p t -> p h t"),
                in_=sq,
            )
            for h in range(HLOC):
                nc.vector.tensor_add(sk[:, h, :], tk[:, h, :], swk[:, h, :])
            nc.scalar.dma_start(
                out=kdr[b].ap()[:, :, s0:s0 + 512].rearrange("h p t -> p h t"),
                in_=sk,
            )


def _vattn_phases(nc, tc, ones_sb, mtri, xt16_d, wv_d, wo_d, cs_d,
                  ss_d, qdr, kdr, vdr, out_d, F32, F32R, F16, EXP):
    import concourse.bass_isa as bass_isa

    hbs = [(b, h) for b in range(B) for h in range(HLOC)]

    # pools, LIFO-ordered: attention pools first (live to the end), V pools
    # on top (released after W3).
    qkv = tc.alloc_tile_pool(name="qkv", bufs=2)
    hold = tc.alloc_tile_pool(name="hold", bufs=1)
    sm = tc.alloc_tile_pool(name="sm", bufs=2)
    w3p = tc.alloc_tile_pool(name="w3p", bufs=1, side="right")   # wo
    w2 = tc.alloc_tile_pool(name="w2", bufs=1, side="right")     # wv
    # with rope allocated first in W1, xtv/vst land in the freed w1/xt1
    # range: the first xg loads are ungated the moment the last QK matmul
    # retires
    xtv = tc.alloc_tile_pool(name="xtv", bufs=4)
    vst = tc.alloc_tile_pool(name="vst", bufs=2)
    # W2 V(b0) uses 512-token chunks; bufs=2 (8 banks) so the next chunk's
    # matmuls overlap the previous chunk's drains.
    psV0 = tc.alloc_tile_pool(name="psV0", bufs=2, space="PSUM")

    wv_sb = w2.tile([128, DKT - 6, 512], F16, tag="wv")
    # first 6 dk-tiles of wv staged in the xtv pool (freed w1/xt1 range,
    # ungated at the last QK matmul): these transfer during the last QK
    # chunk's epilogue so V(b0) starts immediately
    wvs = xtv.tile([128, 6, 512], F16, tag="wvs", bufs=1)
    for dks in (slice(0, 3), slice(3, 6)):
        nc.sync.dma_start(out=wvs[:, dks, :], in_=wv_d.ap()[:, dks, :])

    wo_sb = w3p.tile([128, HLOC, D], F32R, tag="wo")

    def v_weights(c, g):
        # remaining wv tiles (dk 6-31) on the scalar HWDGE queue (sync is
        # dedicated to JIT xg loads), batched one group ahead of use
        if c != 0 or g % 2 != 1:
            return
        if g == 1:
            nc.scalar.dma_start(out=wv_sb[:, 0:2, :], in_=wv_d.ap()[:, 6:8, :])
        lo = 2 * g + 2
        if 8 <= lo < DKT:
            nc.scalar.dma_start(
                out=wv_sb[:, lo - 6:lo - 2, :], in_=wv_d.ap()[:, lo:lo + 4, :]
            )

    def wv_at(dk):
        return wvs[:, dk, :] if dk < 6 else wv_sb[:, dk - 6, :]

    def v_drain(vps, b, s0, t, eng):
        vc = vst.tile([128, 512], F16, name="vc", tag="vc")
        if eng is nc.scalar:
            nc.scalar.copy(vc, vps)
        else:
            eng.tensor_copy(vc, vps)
        nc.gpsimd.dma_start(
            out=vdr[b].ap()[s0 + t * 128:s0 + (t + 1) * 128, :], in_=vc
        )

    def v_chunk0(c):
        # 512-token chunk of batch 0; xg batched 4-dk to halve the HWDGE
        # descriptor-generation load
        s0 = c * 512
        vps = [psV0.tile([128, 512], F32, name=f"v0ps{t}", tag=f"v0{t}")
               for t in range(4)]
        for gq in range(DKT // 4):
            v_weights(c, 2 * gq)
            v_weights(c, 2 * gq + 1)
            xg = xtv.tile([128, 4, 512], F16, name="xg0", tag="xg")
            nc.sync.dma_start(
                out=xg,
                in_=xt16_d.ap()[4 * gq:4 * gq + 4, :, s0:s0 + 512].rearrange(
                    "g p t -> p g t"
                ),
            )
            for gg in range(4):
                dk = 4 * gq + gg
                for t in range(4):
                    nc.tensor.matmul(
                        vps[t], xg[:, gg, t * 128:(t + 1) * 128],
                        wv_at(dk),
                        start=(dk == 0), stop=(dk == DKT - 1),
                    )
        for t in range(4):
            v_drain(vps[t], 0, s0, t, nc.scalar if t % 2 == 0 else nc.vector)

    xg_pre = []

    def load_xg1(i, g2, tag="xg"):
        s0 = i * 256
        if tag == "xgp":
            xg = xtv.tile([128, 8, 256], F16, name="xgp", tag=tag, bufs=3)
        else:
            xg = xtv.tile([128, 8, 256], F16, name="xg", tag=tag)
        nc.sync.dma_start(
            out=xg,
            in_=xt16_d.ap()[8 * g2:8 * g2 + 8, :,
                            S + s0:S + s0 + 256].rearrange(
                "g p t -> p g t"
            ),
        )
        return xg

    def v_chunk1_half(i, psV, vps, hi):
        # half of a 256-token chunk of batch 1 — emitted in two parts around
        # attention units so the PE has V matmuls to run while exps drain
        s0 = i * 256
        if hi == 0:
            vps[:] = [psV.tile([128, 512], F32, name=f"vps{t}", tag=f"v{t}")
                      for t in range(2)]
        for g2 in range(hi * 2, hi * 2 + 2):
            if i == 0 and xg_pre:
                xg = xg_pre.pop(0)
            else:
                xg = load_xg1(i, g2)
            for gg in range(8):
                dk = 8 * g2 + gg
                for t in range(2):
                    nc.tensor.matmul(
                        vps[t], xg[:, gg, t * 128:(t + 1) * 128],
                        wv_at(dk),
                        start=(dk == 0), stop=(dk == DKT - 1),
                    )
        if hi == 1:
            for t in range(2):
                v_drain(vps[t], 1, s0, t, nc.vector)

    # NOTE: loads must be emitted after the spill writes they read — the tile
    # framework only tracks dependencies on already-emitted instructions.
    # Queue plan: sync = JIT xg loads only; gpsimd carries the wait-free
    # qT/kT reloads (their qdr/kdr spills are long done, so the Pool SEQ
    # never head-of-line blocks); scalar/vector take the rest.
    def load_qk(i, eng_q=None, eng_k=None):
        b, h = hbs[i]
        eng_q = eng_q or nc.gpsimd
        eng_k = eng_k or nc.scalar
        qT = qkv.tile([128, S], F16, name=f"qT{i}", tag="qT")
        kT = qkv.tile([128, S], F16, name=f"kT{i}", tag="kT")
        for half in range(2):
            sl = slice(half * (S // 2), (half + 1) * (S // 2))
            eng_q.dma_start(out=qT[:, sl], in_=qdr[b].ap()[h][:, sl])
            eng_k.dma_start(out=kT[:, sl], in_=kdr[b].ap()[h][:, sl])
        return qT, kT

    def load_v(b, hp, eng):
        # fp16 vT for a PAIR of heads: 512-byte contiguous runs keep the
        # descriptor latency multiplier at 1
        vT = qkv.tile([128, KT, 256], F16, name=f"vT{b}{hp}", tag="vT")
        vsrc = vdr[b].ap()[:, hp * 256:(hp + 1) * 256].rearrange(
            "(n p) d -> p n d", p=128
        )
        for half in range(2):
            sl = slice(half * (KT // 2), (half + 1) * (KT // 2))
            eng.dma_start(out=vT[:, sl, :], in_=vsrc[:, sl, :])
        return vT

    qk_tiles = {}
    v_tiles = {}
    hoTs = {}
    attn_ps = [None, None]   # [hops-pool, st-pool]

    def attn_unit(i, qc):
        ps3, ps4 = attn_ps
        b, h = hbs[i]
        qT, kT = qk_tiles[i]
        vT = v_tiles[(b, h // 2)]
        vc0 = (h % 2) * 128
        hoT = hoTs[b]
        qs = qc * 512
        nkt = (qc + 1) * 4
        hops = ps3.tile([128, 512], F32, name="hops", tag="hops")
        acc = sm.tile([128, 512], F32R, name="acc", tag="acc")
        # software-pipelined three deep: scores(kt+1..kt+3) issue before
        # PV(kt) so the PE never head-of-line waits on the exp latency
        pend = []

        def emit_pv(stop):
            pkt, pw, pex = pend.pop(0)
            nc.tensor.matmul(
                hops[:, pw:], vT[:, pkt, vc0:vc0 + 128], pex[:, pw:],
                start=(pkt == 0), stop=stop,
            )

        for kt in range(nkt):
            j = kt - (nkt - 4)
            # fp16 matmuls run 1 cyc/row at any width, so the triangle
            # narrowing is free (no 4x penalty below 256 cols)
            w = 128 * j if j > 0 else 0
            st = ps4.tile([128, 512], F32, name="st", tag="st")
            nc.tensor.matmul(
                st[:, w:], kT[:, kt * 128:(kt + 1) * 128],
                qT[:, qs + w:qs + 512],
                start=True, stop=True,
            )
            if j >= 0:
                nc.vector.tensor_add(st[:, w:w + 128], st[:, w:w + 128], mtri)
            ex = sm.tile([128, 512], F16, name="ex", tag="ex", bufs=5)
            nc.scalar.activation(ex[:, w:], st[:, w:], EXP, scale=ISQRT)
            if len(pend) == 3:
                emit_pv(stop=False)
            if kt == 0:
                nc.vector.tensor_copy(acc, ex)
            else:
                nc.vector.tensor_add(acc[:, w:], acc[:, w:], ex[:, w:])
            pend.append((kt, w, ex))

        def finish():
            # trailing PVs + normalization, emitted by the caller AFTER the
            # next block of independent matmuls: the PE's in-order queue
            # would otherwise head-of-line stall on the last exps
            while pend:
                emit_pv(stop=(len(pend) == 1))
            # softmax denominator on the idle Pool engine (replaces a
            # 512-col PE ones-matmul per unit); output lands broadcast to
            # every partition
            sums = sm.tile([128, 512], F32, name="sums", tag="sums", bufs=2)
            nc.gpsimd.partition_all_reduce(
                sums, acc.bitcast(F32), 128, bass_isa.ReduceOp.add
            )
            rb = sm.tile([128, 512], F32, name="rb", tag="rb", bufs=1)
            nc.vector.reciprocal(rb, sums)
            nc.vector.tensor_mul(hoT[:, h, qs:qs + 512], hops, rb)

        return finish

    # ---- W2: V projection batch 0 ----
    for c in range(4):
        v_chunk0(c)
        if c == 1:
            # first attention loads mid-W2: past the W1->W2 boundary's
            # HWDGE burst, well ahead of W3's first units
            qk_tiles[0] = load_qk(0)
        if c == 2:
            # prefetch the whole first W3 V-chunk's x past the W2->W3
            # boundary
            for g2 in range(4):
                xg_pre.append(load_xg1(0, g2, tag="xgp"))
    # vT reads every token row of vdr[0], so this load may only be emitted
    # once ALL four chunks' drains are emitted (the tile framework orders
    # loads only against already-emitted writes)
    v_tiles[(0, 0)] = load_v(0, 0, nc.scalar)
    psV0.release()

    # ---- W3: V projection batch 1 interleaved with attention(b0) ----
    ps3 = tc.alloc_tile_pool(name="ps3", bufs=2, space="PSUM")   # hops
    ps4 = tc.alloc_tile_pool(name="ps4", bufs=4, space="PSUM")   # st
    psV = tc.alloc_tile_pool(name="psV", bufs=1, space="PSUM")
    attn_ps[:] = [ps3, ps4]
    hoTs[0] = hold.tile([128, HLOC, S], F32R, name="hoT0", tag="hoT", bufs=1)
    fin_prev = None
    for i in range(VCH):
        vps = []
        for u in (2 * i, 2 * i + 1):
            v_chunk1_half(i, psV, vps, u % 2)
            if fin_prev is not None:
                # previous unit's trailing PVs, now covered by the V-half
                # matmuls just emitted
                fin_prev()
            h, qc = divmod(u, QC)
            if qc == 0 and h + 1 < HLOC and h + 1 not in qk_tiles:
                qk_tiles[h + 1] = load_qk(h + 1)
                if h == 1:
                    v_tiles[(0, 1)] = load_v(0, 1, nc.scalar)
            if u == 5:
                # b1 h0 q/k only depend on W1 spills: load early, spread out
                qk_tiles[HLOC] = load_qk(HLOC)
            if u in (2, 6, 10, 13):
                # wo loads spread across W3 so W4's first O-pieces (which
                # touch every wo column) never wait on them
                ho = {2: 0, 6: 1, 10: 2, 13: 3}[u]
                nc.scalar.dma_start(out=wo_sb[:, ho, :],
                                    in_=wo_d.ap()[:, ho, :])
            fin_prev = attn_unit(h, qc)
            if u == 15:
                # (b1,h0) V: only now are ALL vdr[1] spill writes emitted
                # (this half contained the last chunk's drain); emitted after
                # the unit so the brief SEQ wait cannot stall its exps
                v_tiles[(1, 0)] = load_v(1, 0, nc.scalar)
    fin_prev()
    fin_prev = None

    psV.release()
    vst.release()
    xtv.release()
    w2.release()

    # ---- W4: attention(b1) interleaved with O-projection(b0) ----
    ost = tc.alloc_tile_pool(name="ost", bufs=4)
    psO = tc.alloc_tile_pool(name="psO", bufs=2, space="PSUM")

    def o_piece(b, qc, t, half, hoT, three_way, split_store=False):
        # one (t, half) piece: 4 output-column chains + two stores
        c0 = qc * 512 + t * 128
        for pair in range(2):
            ot = ost.tile([128, 2, 512], F16, name="ot", tag="ot")
            for oi in range(2):
                oc = half * 4 + pair * 2 + oi
                if three_way and (pair + oi) % 2 == 1:
                    ops = attn_ps[1].tile([128, 512], F32, name="ops2",
                                          tag="st")
                else:
                    ops = psO.tile([128, 512], F32, name="ops", tag="ops")
                for h in range(HLOC):
                    nc.tensor.matmul(
                        ops, hoT[:, h, c0:c0 + 128],
                        wo_sb[:, h, oc * 512:(oc + 1) * 512],
                        start=(h == 0), stop=(h == HLOC - 1),
                    )
                # drains alternate ACT/DVE (gpsimd cannot access PSUM)
                if (pair * 2 + oi) % 2 == 0:
                    nc.scalar.copy(ot[:, oi, :], ops)
                else:
                    nc.vector.tensor_copy(ot[:, oi, :], ops)
                if split_store:
                    # kernel tail: per-column stores on alternating queues
                    # so the final store after the last drain is small
                    (nc.scalar if oi == 0 else nc.sync).dma_start(
                        out=out_d.ap()[b * S + c0:b * S + c0 + 128,
                                       oc * 512:(oc + 1) * 512],
                        in_=ot[:, oi, :],
                    )
            if not split_store:
                (nc.scalar if (pair + half) % 2 == 0 else nc.sync).dma_start(
                    out=out_d.ap()[b * S + c0:b * S + c0 + 128,
                                   (half * 4 + pair * 2) * 512:
                                   (half * 4 + pair * 2 + 2) * 512],
                    in_=ot,
                )

    def o_block(b, qc, hoT, three_way=False, last=False):
        for t in range(4):
            for half in range(2):
                o_piece(b, qc, t, half, hoT, three_way,
                        split_store=(last and t == 3 and half == 1))

    hold1 = tc.alloc_tile_pool(name="hold1", bufs=1, side="right")
    hoTs[1] = hold1.tile([128, HLOC, S], F32R, name="hoT1", tag="hoT1")
    # O(b0) is entirely ready at W4 start: two of its 32 pieces follow every
    # attention(b1) unit, filling the exp-latency PE slack uniformly
    opieces = [(qc, t, half)
               for qc in range(QC) for t in range(4) for half in range(2)]
    for u in range(HLOC * QC):
        h, qc = divmod(u, QC)
        i = HLOC + h
        if qc == 0 and h + 1 < HLOC and (i + 1) not in qk_tiles:
            qk_tiles[i + 1] = load_qk(i + 1)
            if h == 1:
                v_tiles[(1, 1)] = load_v(1, 1, nc.scalar)
        fin = attn_unit(i, qc)
        for p in (2 * u, 2 * u + 1):
            pqc, pt, phalf = opieces[p]
            o_piece(0, pqc, pt, phalf, hoTs[0], False)
        fin()

    # ---- W5: O-projection(b1) ----
    for qc in range(QC):
        o_block(1, qc, hoTs[1], three_way=True, last=(qc == QC - 1))

    psO.release()
    ost.release()
    hold1.release()
    for p in reversed(attn_ps):
        p.release()
    sm.release()
    hold.release()
    w3p.release()
    qkv.release()


# --------------------------------------------------------------------------
# legacy generic path (non-causal masks)
# --------------------------------------------------------------------------

def _build_legacy():
    import concourse.bacc as bacc
    import concourse.tile as tile
    from concourse import mybir

    F32 = mybir.dt.float32
    F32R = mybir.dt.float32r
    EXP = mybir.ActivationFunctionType.Exp

    nc = bacc.Bacc("TRN2", target_bir_lowering=False, debug=False, num_devices=NC)

    xt_d = nc.dram_tensor("xt", [DKT, 128, TOK], F32R, kind="ExternalInput")
    wq_d = nc.dram_tensor("wq", [128, DKT, 512], F32R, kind="ExternalInput")
    wk_d = nc.dram_tensor("wk", [128, DKT, 512], F32R, kind="ExternalInput")
    wv_d = nc.dram_tensor("wv", [128, DKT, 512], F32R, kind="ExternalInput")
    wo_d = nc.dram_tensor("wo", [128, HLOC, D], F32R, kind="ExternalInput")
    cs_d = nc.dram_tensor("cs", [128, S], F32, kind="ExternalInput")
    ss_d = nc.dram_tensor("ss", [128, S], F32, kind="ExternalInput")
    mk_d = nc.dram_tensor("maskf", [KT, 128, S], F32, kind="ExternalInput")
    out_d = nc.dram_tensor("out", [TOK, D], F32, kind="ExternalOutput")

    qdr = {(h, b): nc.dram_tensor(f"qdr{h}_{b}", [128, S], F32R)
           for h in range(HLOC) for b in range(B)}
    kdr = {(h, b): nc.dram_tensor(f"kdr{h}_{b}", [128, S], F32R)
           for h in range(HLOC) for b in range(B)}
    vdr = {b: nc.dram_tensor(f"vdr{b}", [S, 512], F32R) for b in range(B)}

    with tile.TileContext(nc) as tc:
        with tc.tile_pool(name="consts", bufs=1) as consts:
            ones_sb = consts.tile([128, 1], F32R)
            nc.vector.memset(ones_sb.bitcast(F32), 1.0)

            # Phase 1b: V projection
            with (
                tc.tile_pool(name="w2", bufs=1) as w2,
                tc.tile_pool(name="xt2", bufs=4) as xt2,
                tc.tile_pool(name="vcp", bufs=4) as vcp,
                tc.tile_pool(name="ps2", bufs=2, space="PSUM") as ps2,
            ):
                wv_sb = w2.tile([128, DKT, 512], F32R, tag="wv")
                for ch in range(CH):
                    b, s0 = ch // QC, (ch % QC) * 512
                    vps = [ps2.tile([128, 512], F32, name=f"vps{t}", tag=f"v{t}")
                           for t in range(4)]
                    for dk in range(DKT):
                        if ch == 0:
                            we = nc.scalar if dk % 2 == 0 else nc.sync
                            we.dma_start(out=wv_sb[:, dk, :], in_=wv_d.ap()[:, dk, :])
                        xt = xt2.tile([128, 512], F32R, name="xt", tag="xt")
                        nc.sync.dma_start(
                            out=xt, in_=xt_d.ap()[dk, :, ch * 512:(ch + 1) * 512]
                        )
                        for t in range(4):
                            nc.tensor.matmul(
                                vps[t], xt[:, t * 128:(t + 1) * 128], wv_sb[:, dk, :],
                                start=(dk == 0), stop=(dk == DKT - 1),
                            )
                    for t in range(4):
                        vc = vcp.tile([128, 512], F32R, tag="vc")
                        nc.vector.tensor_copy(vc, vps[t])
                        nc.gpsimd.dma_start(
                            out=vdr[b].ap()[s0 + t * 128:s0 + (t + 1) * 128, :],
                            in_=vc,
                        )

            # Phase 1a: Q,K projections + RoPE
            with (
                tc.tile_pool(name="w1", bufs=1) as w1,
                tc.tile_pool(name="xt1", bufs=4) as xt1,
                tc.tile_pool(name="rope", bufs=2) as rope,
                tc.tile_pool(name="ps1", bufs=1, space="PSUM") as ps1,
            ):
                wq_sb = w1.tile([128, DKT, 512], F32R, tag="wq")
                wk_sb = w1.tile([128, DKT, 512], F32R, tag="wk")
                for ch in range(CH):
                    b, s0 = ch // QC, (ch % QC) * 512
                    cs_sb = rope.tile([128, 512], F32, name="cs_c", tag="cs_c")
                    ss_sb = rope.tile([128, 512], F32, name="ss_c", tag="ss_c")
                    nc.scalar.dma_start(out=cs_sb, in_=cs_d.ap()[:, s0:s0 + 512])
                    nc.scalar.dma_start(out=ss_sb, in_=ss_d.ap()[:, s0:s0 + 512])
                    qps = [ps1.tile([128, 512], F32, name=f"qps{h}", tag=f"q{h}")
                           for h in range(HLOC)]
                    kps = [ps1.tile([128, 512], F32, name=f"kps{h}", tag=f"k{h}")
                           for h in range(HLOC)]
                    for dk in range(DKT):
                        if ch == 0:
                            we = nc.scalar if dk % 2 == 0 else nc.sync
                            wf = nc.sync if dk % 2 == 0 else nc.scalar
                            we.dma_start(out=wq_sb[:, dk, :], in_=wq_d.ap()[:, dk, :])
                            wf.dma_start(out=wk_sb[:, dk, :], in_=wk_d.ap()[:, dk, :])
                        xt = xt1.tile([128, 512], F32R, name="xt", tag="xt")
                        nc.sync.dma_start(
                            out=xt, in_=xt_d.ap()[dk, :, ch * 512:(ch + 1) * 512]
                        )
                        for h in range(HLOC):
                            nc.tensor.matmul(
                                qps[h], wq_sb[:, dk, h * 128:(h + 1) * 128], xt,
                                start=(dk == 0), stop=(dk == DKT - 1),
                            )
                        for h in range(HLOC):
                            nc.tensor.matmul(
                                kps[h], wk_sb[:, dk, h * 128:(h + 1) * 128], xt,
                                start=(dk == 0), stop=(dk == DKT - 1),
                            )
                    work = []
                    for h in range(HLOC):
                        for ps, dst in ((qps[h], qdr), (kps[h], kdr)):
                            pc = rope.tile([128, 512], F32, name="pc", tag="pc",
                                           bufs=4)
                            t1 = rope.tile([128, 512], F32, name="t1", tag="t1",
                                           bufs=8)
                            s1 = rope.tile([128, 512], F32, name="s1", tag="s1",
                                           bufs=2)
                            s1w = rope.tile([128, 512], F32, name="s1w", tag="s1w",
                                            bufs=8)
                            nc.vector.tensor_copy(pc, ps)
                            nc.vector.tensor_mul(t1, pc, cs_sb)
                            nc.vector.tensor_mul(s1, pc, ss_sb)
                            nc.scalar.dma_start(out=s1w[0:64, :], in_=s1[64:128, :])
                            nc.scalar.dma_start(out=s1w[64:128, :], in_=s1[0:64, :])
                            work.append((h, dst, t1, s1w))
                    for h, dst, t1, s1w in work:
                        rr = rope.tile([128, 512], F32R, name="rr", tag="rr", bufs=2)
                        nc.vector.tensor_add(rr, t1, s1w)
                        nc.scalar.dma_start(out=dst[(h, b)].ap()[:, s0:s0 + 512],
                                            in_=rr)

            # Phases 2+3
            hbs = [(b, h) for b in range(B) for h in range(HLOC)]
            with (
                tc.tile_pool(name="qkv", bufs=2) as qkv,
                tc.tile_pool(name="hold", bufs=1) as hold,
                tc.tile_pool(name="smp", bufs=2) as smp,
                tc.tile_pool(name="ps3", bufs=1, space="PSUM") as ps3,
                tc.tile_pool(name="ps4", bufs=3, space="PSUM") as ps4,
            ):
                def load_hb(i):
                    b, h = hbs[i]
                    qT = qkv.tile([128, S], F32R, name=f"qT_{i}", tag="qT")
                    kT = qkv.tile([128, S], F32R, name=f"kT_{i}", tag="kT")
                    vT = qkv.tile([128, KT, 128], F32R, name=f"vT_{i}", tag="vT")
                    vsrc = vdr[b].ap()[:, h * 128:(h + 1) * 128].rearrange(
                        "(n p) d -> p n d", p=128
                    )
                    for j in range(QC):
                        sl = slice(j * 512, (j + 1) * 512)
                        nc.sync.dma_start(out=qT[:, sl], in_=qdr[(h, b)].ap()[:, sl])
                        nc.sync.dma_start(out=kT[:, sl], in_=kdr[(h, b)].ap()[:, sl])
                        nc.sync.dma_start(
                            out=vT[:, j * 4:(j + 1) * 4, :],
                            in_=vsrc[:, j * 4:(j + 1) * 4, :],
                        )
                    return qT, kT, vT

                tiles = {0: load_hb(0)}
                wo_sb = hold.tile([128, HLOC, D], F32R, tag="wo")
                for h in range(HLOC):
                    nc.scalar.dma_start(out=wo_sb[:, h, :], in_=wo_d.ap()[:, h, :])

                hoTs = {}
                for i, (b, h) in enumerate(hbs):
                    if h == 0:
                        hoTs[b] = hold.tile([128, HLOC, S], F32R,
                                            name=f"hoT_{b}", tag=f"hoT{b}")
                    hoT = hoTs[b]
                    if i + 1 < len(hbs):
                        tiles[i + 1] = load_hb(i + 1)
                    qT, kT, vT = tiles.pop(i)
                    for qc in range(QC):
                        qs = qc * 512
                        sums = ps3.tile([1, 512], F32, name="sums", tag="sums")
                        hops = ps3.tile([128, 512], F32, name="hops", tag="hops")
                        for kt in range(KT):
                            st = ps4.tile([128, 512], F32, name="st", tag="st")
                            nc.tensor.matmul(
                                st, kT[:, kt * 128:(kt + 1) * 128],
                                qT[:, qs:qs + 512], start=True, stop=True,
                            )
                            mkt = smp.tile([128, 512], F32, name="mkt", tag="mkt")
                            nc.sync.dma_start(out=mkt, in_=mk_d.ap()[kt, :, qs:qs + 512])
                            nc.vector.tensor_add(st, st, mkt)
                            ex = smp.tile([128, 512], F32R, name="ex", tag="ex",
                                          bufs=4)
                            nc.scalar.activation(ex, st, EXP, scale=ISQRT)
                            nc.tensor.matmul(sums, ones_sb, ex, start=(kt == 0),
                                             stop=(kt == KT - 1))
                            nc.tensor.matmul(hops, vT[:, kt, :], ex, start=(kt == 0),
                                             stop=(kt == KT - 1))
                        recip = smp.tile([1, 512], F32, name="recip", tag="recip")
                        nc.vector.reciprocal(recip, sums)
                        bc = smp.tile([128, 512], F32, name="bc", tag="bc")
                        nc.gpsimd.partition_broadcast(bc, recip)
                        nc.vector.tensor_mul(hoT[:, h, qs:qs + 512], hops, bc)

                for b in range(B):
                    with (
                        tc.tile_pool(name=f"oc{b}", bufs=3) as ocp,
                        tc.tile_pool(name=f"ps5{b}", bufs=3, space="PSUM") as ps5,
                    ):
                        for t in range(S // 128):
                            for oc in range(D // 512):
                                ops = ps5.tile([128, 512], F32, name="ops", tag="ops")
                                for h in range(HLOC):
                                    nc.tensor.matmul(
                                        ops, hoTs[b][:, h, t * 128:(t + 1) * 128],
                                        wo_sb[:, h, oc * 512:(oc + 1) * 512],
                                        start=(h == 0), stop=(h == HLOC - 1),
                                    )
                                ot = ocp.tile([128, 512], F32, name="ot", tag="ot")
                                nc.vector.tensor_copy(ot, ops)
                                nc.scalar.dma_start(
                                    out=out_d.ap()[
                                        b * S + t * 128:b * S + (t + 1) * 128,
                                        oc * 512:(oc + 1) * 512,
                                    ],
                                    in_=ot,
                                )

    nc.compile()
    return nc


def _get_nc(causal: bool):
    if causal not in _CACHE:
        _CACHE[causal] = _build_causal() if causal else _build_legacy()
    return _CACHE[causal]


def _host_prep(x, wq, wk, wv, wo, freqs_cos, freqs_sin, mask):
    """Build per-core input maps."""
    x2 = np.ascontiguousarray(x.reshape(TOK, D).T)          # [D, TOK]
    xt = x2.reshape(DKT, 128, TOK)

    cs = np.concatenate([freqs_cos.T, freqs_cos.T], axis=0).astype(np.float32)
    ss = np.concatenate([freqs_sin.T, -freqs_sin.T], axis=0).astype(np.float32)

    m2 = np.asarray(mask, dtype=np.float32).reshape(S, S)
    tril = np.tril(np.ones((S, S), dtype=bool))
    causal = bool(np.all(m2[tril] == 0.0) and np.all(m2[~tril] <= -1e8))
    if causal:
        mk = np.ascontiguousarray(m2[:128, :128].T)         # [k,q] triangle
    else:
        mk = np.ascontiguousarray(m2.T.reshape(KT, 128, S))

    # per-head column permutation: evens then odds (RoPE rotate-half form)
    perm = np.concatenate([np.arange(0, HD, 2), np.arange(1, HD, 2)])

    xt16 = xt.astype(np.float16)

    in_maps = []
    for c in range(NC):
        cols = np.concatenate(
            [(4 * c + h) * HD + perm for h in range(HLOC)]
        )
        wq_c = np.ascontiguousarray(
            wq[:, cols].reshape(DKT, 128, 512).transpose(1, 0, 2)
        )
        wk_c = np.ascontiguousarray(
            wk[:, cols].reshape(DKT, 128, 512).transpose(1, 0, 2)
        )
        vcols = np.arange(4 * c * HD, 4 * (c + 1) * HD)
        wv_c = np.ascontiguousarray(
            wv[:, vcols].reshape(DKT, 128, 512).transpose(1, 0, 2)
        )
        wo_c = np.ascontiguousarray(
            wo[vcols, :].reshape(HLOC, 128, D).transpose(1, 0, 2)
        )
        if causal:
            wq_c = wq_c.astype(np.float16)
            wk_c = wk_c.astype(np.float16)
            wv_c = wv_c.astype(np.float16)
            m = {
                "xt16": xt16, "wq": wq_c, "wk": wk_c, "wv": wv_c,
                "wo": wo_c, "cs": cs, "ss": ss, "mtri": mk,
            }
        else:
            m = {
                "xt": xt, "wq": wq_c, "wk": wk_c, "wv": wv_c, "wo": wo_c,
                "cs": cs, "ss": ss, "maskf": mk,
            }
        in_maps.append(m)
    return in_maps, causal


def kernel(x, wq, wk, wv, wo, freqs_cos, freqs_sin, mask, **_unused):
    from concourse.bass_utils import run_bass_kernel_spmd

    x = np.asarray(x, dtype=np.float32)
    wq = np.asarray(wq, dtype=np.float32)
    wk = np.asarray(wk, dtype=np.float32)
    wv = np.asarray(wv, dtype=np.float32)
    wo = np.asarray(wo, dtype=np.float32)
    freqs_cos = np.asarray(freqs_cos, dtype=np.float32)
    freqs_sin = np.asarray(freqs_sin, dtype=np.float32)

    in_maps, causal = _host_prep(x, wq, wk, wv, wo, freqs_cos, freqs_sin, mask)
    nc = _get_nc(causal)
    res = run_bass_kernel_spmd(nc, in_maps, list(range(NC)))
    out = res.results[0]["out"].astype(np.float32)
    for c in range(1, NC):
        out = out + res.results[c]["out"].astype(np.float32)
    return out.reshape(B, S, D).astype(np.float32)



# revision 74
# speedup vs baseline: 1.0030x; 1.0024x over previous
"""Trainium2 Bass kernel for multi-head causal attention with RoPE.

Model (per reference):
  B=2, S=2048, D=4096, H=32 heads, HD=128.
  out = softmax(rope(x@wq) @ rope(x@wk)^T / sqrt(HD) + mask) @ (x@wv) @ wo

Sharding: tensor-parallel over heads. Core c in 0..7 owns heads 4c..4c+3:
wq/wk/wv column-sharded, wo row-sharded; each core produces a full-shape
partial output and the host sums the 8 partials (the all-reduce).

Precision: all projections and attention matmuls run fp16 (same 1 cyc/row
as fp32r on the PE but half the DMA traffic, and no 4x penalty on
sub-256-col tiles, so the triangle-narrowed diagonals are free). PSUM
accumulation stays fp32; rope combines in fp32 and rounds once to fp16 at
the spill; the O-projection (hoT @ wo) stays fp32r. Measured rel err
1.72e-2 vs the 2e-2 gate (deterministic: fixed seed).

Causal fast path — 5 overlapped windows keeping the PE stall-free:
  W1  QK projections (512-token chunks, 8 PSUM banks), RoPE epilogue with
      PSUM drains split across ACT+DVE, fp16 partition swaps on the SWDGE
      queue, batched fp16 spills. Weights trickle in 2-dk batches one
      iteration ahead; xt tiles prefetch across chunk boundaries. The rope
      pool is allocated FIRST so the W2 pools land in w1/xt1's address
      range, whose last readers are the final matmuls — not the epilogue
      spills — ungating W2's loads ~20us earlier.
  W2  V projection batch 0 (512-token chunks, 2x4 PSUM banks); first
      attention loads and W3 x prefetches mid-window.
  W3  V projection batch 1 interleaved with attention(b0); wo loads
      spread across the window on the scalar queue.
  W4  attention(b1) interleaved with O-projection(b0), two o_pieces after
      every unit.
  W5  O-projection(b1), three-way PSUM rotation, split tail stores.
Attention per (h,qc) unit: fp16 scores with triangle-narrowed diagonal
tiles, exp on ACT to fp16, PV accumulated in PSUM, software-pipelined
three deep (scores kt+1..kt+3 issue before PV kt) with the trailing PVs
emitted by the caller AFTER the next block of independent matmuls (the
PE's in-order queue would otherwise head-of-line stall on the last exps).
Softmax denominator: fp32 DVE accumulation of ex, reduced across
partitions by gpsimd partition_all_reduce (replaces a 512-col PE
ones-matmul per unit), reciprocal+mul on DVE.
Queue plan: sync = JIT x loads only; scalar = weights/spills/attention
loads; SWDGE (gpsimd) = wait-free loads (wk, rope consts, qT reloads,
swaps) — a waiting SWDGE/HWDGE dma head-of-line blocks its engine's whole
sequencer, and HWDGE descriptor generation (~0.63us/DMA) is serialized
across the scalar+sync queues, so boundary bursts are kept off it.
"""

import sys

if "/opt/trn_rl_repo" not in sys.path:
    sys.path.insert(0, "/opt/trn_rl_repo")

import math

import numpy as np

B, S, D, H = 2, 2048, 4096, 32
HD = D // H          # 128
HLOC = 4             # heads per core
NC = 8               # cores
TOK = B * S          # 4096
CH = TOK // 512      # 8 token chunks of 512 (QK phase)
VCH = 8              # 256-token chunks per batch (V phase)
DKT = D // 128       # 32 contraction tiles
QC = S // 512        # 4 q-chunks per sequence
KT = S // 128        # 16 k-tiles per sequence
ISQRT = 1.0 / math.sqrt(HD)

_CACHE = {}


# --------------------------------------------------------------------------
# causal fast path
# --------------------------------------------------------------------------

def _build_causal(nrep: int = 1):
    import concourse.bacc as bacc
    import concourse.tile as tile
    from concourse import mybir

    F32 = mybir.dt.float32
    F32R = mybir.dt.float32r
    F16 = mybir.dt.float16
    EXP = mybir.ActivationFunctionType.Exp

    nc = bacc.Bacc("TRN2", target_bir_lowering=False, debug=False, num_devices=NC)

    xt16_d = nc.dram_tensor("xt16", [DKT, 128, TOK], F16, kind="ExternalInput")
    wq_d = nc.dram_tensor("wq", [128, DKT, 512], F16, kind="ExternalInput")
    wk_d = nc.dram_tensor("wk", [128, DKT, 512], F16, kind="ExternalInput")
    wv_d = nc.dram_tensor("wv", [128, DKT, 512], F16, kind="ExternalInput")
    wo_d = nc.dram_tensor("wo", [128, HLOC, D], F32R, kind="ExternalInput")
    cs_d = nc.dram_tensor("cs", [128, S], F32, kind="ExternalInput")
    ss_d = nc.dram_tensor("ss", [128, S], F32, kind="ExternalInput")
    mt_d = nc.dram_tensor("mtri", [128, 128], F32, kind="ExternalInput")
    # fp16 partial output: halves the 64MB store, host sums in fp32
    out_d = nc.dram_tensor("out", [TOK, D], F16, kind="ExternalOutput")

    # DRAM scratch for projected Q/K/V, all spilled in fp16: halves the
    # spill+reload traffic and the attention matmuls run fp16 (1 cyc/row at
    # any width, so the narrow diagonal tiles are free)
    qdr = {b: nc.dram_tensor(f"qdr{b}", [HLOC, 128, S], F16) for b in range(B)}
    kdr = {b: nc.dram_tensor(f"kdr{b}", [HLOC, 128, S], F16) for b in range(B)}
    vdr = {b: nc.dram_tensor(f"vdr{b}", [S, 512], F16) for b in range(B)}

    with tile.TileContext(nc) as tc:
        with tc.tile_pool(name="consts", bufs=1) as consts:
            # all-ones [128,128] lhsT: the sums matmul broadcasts the column
            # sums to every partition (same 512-column cost), so no separate
            # partition-broadcast is needed for the normalization
            ones_sb = consts.tile([128, 128], F32R)
            nc.vector.memset(ones_sb.bitcast(F32), 1.0)
            mtri = consts.tile([128, 128], F32, name="mtri")
            # SWDGE queue: keeps the startup HWDGE/scalar path clear for
            # the first weight batches
            nc.gpsimd.dma_start(out=mtri, in_=mt_d.ap())
            for _ in range(nrep):
                _qk_phase(nc, tc, xt16_d, wq_d, wk_d, cs_d, ss_d, qdr, kdr,
                          F32, F32R, F16)
                _vattn_phases(nc, tc, ones_sb, mtri, xt16_d, wv_d,
                              wo_d, cs_d, ss_d, qdr, kdr, vdr, out_d,
                              F32, F32R, F16, EXP)

    nc.compile()
    return nc


def _qk_phase(nc, tc, xt_d, wq_d, wk_d, cs_d, ss_d, qdr, kdr, F32, F32R, F16):
    """Q,K projections emitted in transposed [HD, tok] layout with RoPE.

    Entirely fp16 on the PE (weights + x): same 1 cyc/row as fp32r but half
    the DMA traffic, which un-saturates the bus during chunk 0 (weights + x
    + rope constants used to exceed the 360GB/s budget there).
    """
    with (
        # rope FIRST: its last readers are the final epilogue spill DMAs
        # (~20us after the last matmul), so any W2 pool landing in its
        # address range is gated that long. With rope at the bottom, the
        # W2 x/weight pools land in w1/xt1's range, whose last readers are
        # the final QK matmuls — W2's loads start immediately.
        tc.tile_pool(name="rope", bufs=1) as rope,
        tc.tile_pool(name="w1", bufs=1) as w1,
        tc.tile_pool(name="xt1", bufs=3) as xt1,
        tc.tile_pool(name="ps1", bufs=1, space="PSUM") as ps1,
    ):
        wq_sb = w1.tile([128, DKT, 512], F16, tag="wq")
        wk_sb = w1.tile([128, DKT, 512], F16, tag="wk")

        def load_xt(ch, g):
            xt = xt1.tile([128, 2, 512], F16, name="xt", tag="xt", bufs=6)
            nc.sync.dma_start(
                out=xt,
                in_=xt_d.ap()[2 * g:2 * g + 2, :,
                              ch * 512:(ch + 1) * 512].rearrange(
                    "g p t -> p g t"
                ),
            )
            return xt

        xt_pre = {}
        for ch in range(CH):
            b, s0 = ch // QC, (ch % QC) * 512
            cs_sb = rope.tile([128, 512], F32, name="cs_c", tag="cs_c", bufs=2)
            ss_sb = rope.tile([128, 512], F32, name="ss_c", tag="ss_c", bufs=2)
            qps = [ps1.tile([128, 512], F32, name=f"qps{h}", tag=f"q{h}")
                   for h in range(HLOC)]
            kps = [ps1.tile([128, 512], F32, name=f"kps{h}", tag=f"k{h}")
                   for h in range(HLOC)]
            for g in range(DKT // 2):
                # xt first: the g==0 load is on the critical path to the
                # very first matmul; chunk-boundary tiles were prefetched
                xt = xt_pre.pop(g, None) or load_xt(ch, g)
                if ch == 0:
                    # weights trickle in 2-dk batches one iteration ahead of
                    # use: per-g bus demand stays under the PE's consumption
                    # rate, so neither stream ever starves the other
                    if g == 0:
                        # wk(0:2) on scalar: the Pool queue starts with the
                        # framework's init memsets, which would delay the
                        # first k-matmuls by ~2us
                        nc.scalar.dma_start(
                            out=wq_sb[:, 0:2, :], in_=wq_d.ap()[:, 0:2, :]
                        )
                        nc.scalar.dma_start(
                            out=wk_sb[:, 0:2, :], in_=wk_d.ap()[:, 0:2, :]
                        )
                        nc.scalar.dma_start(
                            out=wq_sb[:, 2:4, :], in_=wq_d.ap()[:, 2:4, :]
                        )
                        nc.gpsimd.dma_start(
                            out=wk_sb[:, 2:4, :], in_=wk_d.ap()[:, 2:4, :]
                        )
                    elif g < 15:
                        lo = 2 * g + 2
                        nc.scalar.dma_start(
                            out=wq_sb[:, lo:lo + 2, :],
                            in_=wq_d.ap()[:, lo:lo + 2, :],
                        )
                        nc.gpsimd.dma_start(
                            out=wk_sb[:, lo:lo + 2, :],
                            in_=wk_d.ap()[:, lo:lo + 2, :],
                        )
                if g == 8:
                    # rope constants mid-loop on the SWDGE queue: off the
                    # startup critical path, well ahead of the epilogue
                    nc.gpsimd.dma_start(out=cs_sb,
                                        in_=cs_d.ap()[:, s0:s0 + 512])
                    nc.gpsimd.dma_start(out=ss_sb,
                                        in_=ss_d.ap()[:, s0:s0 + 512])
                if ch + 1 < CH and g in (13, 14):
                    # prefetch the next chunk's first xt tiles past the
                    # epilogue's DMA burst at the boundary
                    xt_pre[g - 13] = load_xt(ch + 1, g - 13)
                for gg in range(2):
                    dk = 2 * g + gg
                    for h in range(HLOC):
                        nc.tensor.matmul(
                            qps[h], wq_sb[:, dk, h * 128:(h + 1) * 128],
                            xt[:, gg, :],
                            start=(dk == 0), stop=(dk == DKT - 1),
                        )
                    for h in range(HLOC):
                        nc.tensor.matmul(
                            kps[h], wk_sb[:, dk, h * 128:(h + 1) * 128],
                            xt[:, gg, :],
                            start=(dk == 0), stop=(dk == DKT - 1),
                        )
            # epilogue pass 1: drain all 8 PSUM banks first (frees banks for
            # the next chunk) — copies split across ACT and DVE
            pcs = []
            for i, ps in enumerate(qps + kps):
                pc = rope.tile([128, 512], F32, name="pc", tag="pc", bufs=4)
                if i % 2 == 0:
                    nc.scalar.copy(pc, ps)
                else:
                    nc.vector.tensor_copy(pc, ps)
                pcs.append(pc)
            # pass 2: rope products; s-terms in fp16 (halves the swap DMAs),
            # cos-terms kept fp32 so the final value rounds only twice
            tq = rope.tile([128, HLOC, 512], F32, name="tq", tag="tq")
            tk = rope.tile([128, HLOC, 512], F32, name="tk", tag="tk")
            sq = rope.tile([128, HLOC, 512], F16, name="sq", tag="sq")
            sk = rope.tile([128, HLOC, 512], F16, name="sk", tag="sk")
            swq = rope.tile([128, HLOC, 512], F16, name="swq", tag="swq")
            swk = rope.tile([128, HLOC, 512], F16, name="swk", tag="swk")
            for h in range(HLOC):
                nc.vector.tensor_mul(sq[:, h, :], pcs[h], ss_sb)
                nc.vector.tensor_mul(sk[:, h, :], pcs[HLOC + h], ss_sb)
            # partition swaps on the SWDGE queue: keeps the boundary burst
            # off the serialized HWDGE descriptor generator
            nc.gpsimd.dma_start(out=swq[0:64], in_=sq[64:128])
            nc.gpsimd.dma_start(out=swq[64:128], in_=sq[0:64])
            nc.gpsimd.dma_start(out=swk[0:64], in_=sk[64:128])
            nc.gpsimd.dma_start(out=swk[64:128], in_=sk[0:64])
            for h in range(HLOC):
                nc.vector.tensor_mul(tq[:, h, :], pcs[h], cs_sb)
                nc.vector.tensor_mul(tk[:, h, :], pcs[HLOC + h], cs_sb)
            # pass 3: combine into fp16 + batched spill; the fp16 outputs
            # reuse the sq/sk tiles whose last readers are the swap DMAs
            for h in range(HLOC):
                nc.vector.tensor_add(sq[:, h, :], tq[:, h, :], swq[:, h, :])
            nc.scalar.dma_start(
                out=qdr[b].ap()[:, :, s0:s0 + 512].rearrange("h p t -> p h t"),
                in_=sq,
            )
            for h in range(HLOC):
                nc.vector.tensor_add(sk[:, h, :], tk[:, h, :], swk[:, h, :])
            nc.scalar.dma_start(
                out=kdr[b].ap()[:, :, s0:s0 + 512].rearrange("h p t -> p h t"),
                in_=sk,
            )


def _vattn_phases(nc, tc, ones_sb, mtri, xt16_d, wv_d, wo_d, cs_d,
                  ss_d, qdr, kdr, vdr, out_d, F32, F32R, F16, EXP):
    import concourse.bass_isa as bass_isa

    hbs = [(b, h) for b in range(B) for h in range(HLOC)]

    # pools, LIFO-ordered: attention pools first (live to the end), V pools
    # on top (released after W3).
    qkv = tc.alloc_tile_pool(name="qkv", bufs=2)
    hold = tc.alloc_tile_pool(name="hold", bufs=1)
    sm = tc.alloc_tile_pool(name="sm", bufs=2)
    w3p = tc.alloc_tile_pool(name="w3p", bufs=1, side="right")   # wo
    w2 = tc.alloc_tile_pool(name="w2", bufs=1, side="right")     # wv
    # with rope allocated first in W1, xtv/vst land in the freed w1/xt1
    # range: the first xg loads are ungated the moment the last QK matmul
    # retires
    xtv = tc.alloc_tile_pool(name="xtv", bufs=5)
    vst = tc.alloc_tile_pool(name="vst", bufs=2)
    # W2 V(b0) uses 512-token chunks; bufs=2 (8 banks) so the next chunk's
    # matmuls overlap the previous chunk's drains.
    psV0 = tc.alloc_tile_pool(name="psV0", bufs=2, space="PSUM")

    wv_sb = w2.tile([128, DKT - 6, 512], F16, tag="wv")
    # first 6 dk-tiles of wv staged in the xtv pool (freed w1/xt1 range,
    # ungated at the last QK matmul): these transfer during the last QK
    # chunk's epilogue so V(b0) starts immediately
    wvs = xtv.tile([128, 6, 512], F16, tag="wvs", bufs=1)
    for dks in (slice(0, 3), slice(3, 6)):
        nc.sync.dma_start(out=wvs[:, dks, :], in_=wv_d.ap()[:, dks, :])

    wo_sb = w3p.tile([128, HLOC, D], F32R, tag="wo")

    def v_weights(c, g):
        # remaining wv tiles (dk 6-31) on the scalar HWDGE queue (sync is
        # dedicated to JIT xg loads), batched one group ahead of use
        if c != 0 or g % 2 != 1:
            return
        if g == 1:
            nc.scalar.dma_start(out=wv_sb[:, 0:2, :], in_=wv_d.ap()[:, 6:8, :])
        lo = 2 * g + 2
        if 8 <= lo < DKT:
            nc.scalar.dma_start(
                out=wv_sb[:, lo - 6:lo - 2, :], in_=wv_d.ap()[:, lo:lo + 4, :]
            )

    def wv_at(dk):
        return wvs[:, dk, :] if dk < 6 else wv_sb[:, dk - 6, :]

    def v_drain(vps, b, s0, t, eng):
        vc = vst.tile([128, 512], F16, name="vc", tag="vc")
        if eng is nc.scalar:
            nc.scalar.copy(vc, vps)
        else:
            eng.tensor_copy(vc, vps)
        nc.gpsimd.dma_start(
            out=vdr[b].ap()[s0 + t * 128:s0 + (t + 1) * 128, :], in_=vc
        )

    def v_chunk0(c):
        # 512-token chunk of batch 0; xg batched 4-dk to halve the HWDGE
        # descriptor-generation load
        s0 = c * 512
        vps = [psV0.tile([128, 512], F32, name=f"v0ps{t}", tag=f"v0{t}")
               for t in range(4)]
        for gq in range(DKT // 4):
            v_weights(c, 2 * gq)
            v_weights(c, 2 * gq + 1)
            xg = xtv.tile([128, 4, 512], F16, name="xg0", tag="xg")
            nc.sync.dma_start(
                out=xg,
                in_=xt16_d.ap()[4 * gq:4 * gq + 4, :, s0:s0 + 512].rearrange(
                    "g p t -> p g t"
                ),
            )
            for gg in range(4):
                dk = 4 * gq + gg
                for t in range(4):
                    nc.tensor.matmul(
                        vps[t], xg[:, gg, t * 128:(t + 1) * 128],
                        wv_at(dk),
                        start=(dk == 0), stop=(dk == DKT - 1),
                    )
        for t in range(4):
            v_drain(vps[t], 0, s0, t, nc.scalar if t % 2 == 0 else nc.vector)

    xg_pre = []

    def load_xg1(i, g2, tag="xg"):
        s0 = i * 256
        if tag == "xgp":
            xg = xtv.tile([128, 8, 256], F16, name="xgp", tag=tag, bufs=2)
        else:
            xg = xtv.tile([128, 8, 256], F16, name="xg", tag=tag)
        nc.sync.dma_start(
            out=xg,
            in_=xt16_d.ap()[8 * g2:8 * g2 + 8, :,
                            S + s0:S + s0 + 256].rearrange(
                "g p t -> p g t"
            ),
        )
        return xg

    def v_chunk1_half(i, psV, vps, hi):
        # half of a 256-token chunk of batch 1 — emitted in two parts around
        # attention units so the PE has V matmuls to run while exps drain
        s0 = i * 256
        if hi == 0:
            vps[:] = [psV.tile([128, 512], F32, name=f"vps{t}", tag=f"v{t}")
                      for t in range(2)]
        for g2 in range(hi * 2, hi * 2 + 2):
            if i == 0 and xg_pre:
                xg = xg_pre.pop(0)
            else:
                xg = load_xg1(i, g2)
            for gg in range(8):
                dk = 8 * g2 + gg
                for t in range(2):
                    nc.tensor.matmul(
                        vps[t], xg[:, gg, t * 128:(t + 1) * 128],
                        wv_at(dk),
                        start=(dk == 0), stop=(dk == DKT - 1),
                    )
        if hi == 1:
            for t in range(2):
                v_drain(vps[t], 1, s0, t, nc.vector)

    # NOTE: loads must be emitted after the spill writes they read — the tile
    # framework only tracks dependencies on already-emitted instructions.
    # Queue plan: sync = JIT xg loads only; gpsimd carries the wait-free
    # qT/kT reloads (their qdr/kdr spills are long done, so the Pool SEQ
    # never head-of-line blocks); scalar/vector take the rest.
    def load_qk(i, eng_q=None, eng_k=None):
        b, h = hbs[i]
        eng_q = eng_q or nc.gpsimd
        eng_k = eng_k or nc.scalar
        qT = qkv.tile([128, S], F16, name=f"qT{i}", tag="qT")
        kT = qkv.tile([128, S], F16, name=f"kT{i}", tag="kT")
        for half in range(2):
            sl = slice(half * (S // 2), (half + 1) * (S // 2))
            eng_q.dma_start(out=qT[:, sl], in_=qdr[b].ap()[h][:, sl])
            eng_k.dma_start(out=kT[:, sl], in_=kdr[b].ap()[h][:, sl])
        return qT, kT

    def load_v(b, hp, eng):
        # fp16 vT for a PAIR of heads: 512-byte contiguous runs keep the
        # descriptor latency multiplier at 1
        vT = qkv.tile([128, KT, 256], F16, name=f"vT{b}{hp}", tag="vT")
        vsrc = vdr[b].ap()[:, hp * 256:(hp + 1) * 256].rearrange(
            "(n p) d -> p n d", p=128
        )
        for half in range(2):
            sl = slice(half * (KT // 2), (half + 1) * (KT // 2))
            eng.dma_start(out=vT[:, sl, :], in_=vsrc[:, sl, :])
        return vT

    qk_tiles = {}
    v_tiles = {}
    hoTs = {}
    attn_ps = [None, None]   # [hops-pool, st-pool]

    def attn_unit(i, qc):
        ps3, ps4 = attn_ps
        b, h = hbs[i]
        qT, kT = qk_tiles[i]
        vT = v_tiles[(b, h // 2)]
        vc0 = (h % 2) * 128
        hoT = hoTs[b]
        qs = qc * 512
        nkt = (qc + 1) * 4
        hops = ps3.tile([128, 512], F32, name="hops", tag="hops")
        acc = sm.tile([128, 512], F32R, name="acc", tag="acc")
        # software-pipelined three deep: scores(kt+1..kt+3) issue before
        # PV(kt) so the PE never head-of-line waits on the exp latency
        pend = []

        def emit_pv(stop):
            pkt, pw, pex = pend.pop(0)
            nc.tensor.matmul(
                hops[:, pw:], vT[:, pkt, vc0:vc0 + 128], pex[:, pw:],
                start=(pkt == 0), stop=stop,
            )

        for kt in range(nkt):
            j = kt - (nkt - 4)
            # fp16 matmuls run 1 cyc/row at any width, so the triangle
            # narrowing is free (no 4x penalty below 256 cols)
            w = 128 * j if j > 0 else 0
            st = ps4.tile([128, 512], F32, name="st", tag="st")
            nc.tensor.matmul(
                st[:, w:], kT[:, kt * 128:(kt + 1) * 128],
                qT[:, qs + w:qs + 512],
                start=True, stop=True,
            )
            if j >= 0:
                nc.vector.tensor_add(st[:, w:w + 128], st[:, w:w + 128], mtri)
            ex = sm.tile([128, 512], F16, name="ex", tag="ex", bufs=5)
            nc.scalar.activation(ex[:, w:], st[:, w:], EXP, scale=ISQRT)
            if len(pend) == 3:
                emit_pv(stop=False)
            if kt == 0:
                nc.vector.tensor_copy(acc, ex)
            else:
                nc.vector.tensor_add(acc[:, w:], acc[:, w:], ex[:, w:])
            pend.append((kt, w, ex))

        def finish():
            # trailing PVs + normalization, emitted by the caller AFTER the
            # next block of independent matmuls: the PE's in-order queue
            # would otherwise head-of-line stall on the last exps
            while pend:
                emit_pv(stop=(len(pend) == 1))
            # softmax denominator on the idle Pool engine (replaces a
            # 512-col PE ones-matmul per unit); output lands broadcast to
            # every partition
            sums = sm.tile([128, 512], F32, name="sums", tag="sums", bufs=2)
            nc.gpsimd.partition_all_reduce(
                sums, acc.bitcast(F32), 128, bass_isa.ReduceOp.add
            )
            rb = sm.tile([128, 512], F32, name="rb", tag="rb", bufs=1)
            nc.vector.reciprocal(rb, sums)
            nc.vector.tensor_mul(hoT[:, h, qs:qs + 512], hops, rb)

        return finish

    # ---- W2: V projection batch 0 ----
    for c in range(4):
        v_chunk0(c)
        if c == 1:
            # first attention loads mid-W2: past the W1->W2 boundary's
            # HWDGE burst, well ahead of W3's first units
            qk_tiles[0] = load_qk(0)
        if c == 2:
            # prefetch the first W3 V-half's x past the W2->W3 boundary
            xg_pre.append(load_xg1(0, 0, tag="xgp"))
            xg_pre.append(load_xg1(0, 1, tag="xgp"))
    # vT reads every token row of vdr[0], so this load may only be emitted
    # once ALL four chunks' drains are emitted (the tile framework orders
    # loads only against already-emitted writes)
    v_tiles[(0, 0)] = load_v(0, 0, nc.scalar)
    psV0.release()

    # ---- W3: V projection batch 1 interleaved with attention(b0) ----
    ps3 = tc.alloc_tile_pool(name="ps3", bufs=2, space="PSUM")   # hops
    ps4 = tc.alloc_tile_pool(name="ps4", bufs=4, space="PSUM")   # st
    psV = tc.alloc_tile_pool(name="psV", bufs=1, space="PSUM")
    attn_ps[:] = [ps3, ps4]
    hoTs[0] = hold.tile([128, HLOC, S], F32R, name="hoT0", tag="hoT", bufs=1)
    fin_prev = None
    for i in range(VCH):
        vps = []
        for u in (2 * i, 2 * i + 1):
            v_chunk1_half(i, psV, vps, u % 2)
            if fin_prev is not None:
                # previous unit's trailing PVs, now covered by the V-half
                # matmuls just emitted
                fin_prev()
            h, qc = divmod(u, QC)
            if qc == 0 and h + 1 < HLOC and h + 1 not in qk_tiles:
                qk_tiles[h + 1] = load_qk(h + 1)
                if h == 1:
                    v_tiles[(0, 1)] = load_v(0, 1, nc.scalar)
            if u == 5:
                # b1 h0 q/k only depend on W1 spills: load early, spread out
                qk_tiles[HLOC] = load_qk(HLOC)
            if u in (2, 6, 10, 13):
                # wo loads spread across W3 so W4's first O-pieces (which
                # touch every wo column) never wait on them
                ho = {2: 0, 6: 1, 10: 2, 13: 3}[u]
                nc.scalar.dma_start(out=wo_sb[:, ho, :],
                                    in_=wo_d.ap()[:, ho, :])
            fin_prev = attn_unit(h, qc)
            if u == 15:
                # (b1,h0) V: only now are ALL vdr[1] spill writes emitted
                # (this half contained the last chunk's drain); emitted after
                # the unit so the brief SEQ wait cannot stall its exps
                v_tiles[(1, 0)] = load_v(1, 0, nc.scalar)
    fin_prev()
    fin_prev = None

    psV.release()
    vst.release()
    xtv.release()
    w2.release()

    # ---- W4: attention(b1) interleaved with O-projection(b0) ----
    ost = tc.alloc_tile_pool(name="ost", bufs=4)
    psO = tc.alloc_tile_pool(name="psO", bufs=2, space="PSUM")

    def o_piece(b, qc, t, half, hoT, three_way, split_store=False):
        # one (t, half) piece: 4 output-column chains + two stores
        c0 = qc * 512 + t * 128
        for pair in range(2):
            ot = ost.tile([128, 2, 512], F16, name="ot", tag="ot")
            for oi in range(2):
                oc = half * 4 + pair * 2 + oi
                if three_way and (pair + oi) % 2 == 1:
                    ops = attn_ps[1].tile([128, 512], F32, name="ops2",
                                          tag="st")
                else:
                    ops = psO.tile([128, 512], F32, name="ops", tag="ops")
                for h in range(HLOC):
                    nc.tensor.matmul(
                        ops, hoT[:, h, c0:c0 + 128],
                        wo_sb[:, h, oc * 512:(oc + 1) * 512],
                        start=(h == 0), stop=(h == HLOC - 1),
                    )
                # drains alternate ACT/DVE (gpsimd cannot access PSUM)
                if (pair * 2 + oi) % 2 == 0:
                    nc.scalar.copy(ot[:, oi, :], ops)
                else:
                    nc.vector.tensor_copy(ot[:, oi, :], ops)
                if split_store:
                    # kernel tail: per-column stores on alternating queues
                    # so the final store after the last drain is small
                    (nc.scalar if oi == 0 else nc.sync).dma_start(
                        out=out_d.ap()[b * S + c0:b * S + c0 + 128,
                                       oc * 512:(oc + 1) * 512],
                        in_=ot[:, oi, :],
                    )
            if not split_store:
                (nc.scalar if (pair + half) % 2 == 0 else nc.sync).dma_start(
                    out=out_d.ap()[b * S + c0:b * S + c0 + 128,
                                   (half * 4 + pair * 2) * 512:
                                   (half * 4 + pair * 2 + 2) * 512],
                    in_=ot,
                )

    def o_block(b, qc, hoT, three_way=False, last=False):
        for t in range(4):
            for half in range(2):
                o_piece(b, qc, t, half, hoT, three_way,
                        split_store=(last and t == 3 and half == 1))

    hold1 = tc.alloc_tile_pool(name="hold1", bufs=1, side="right")
    hoTs[1] = hold1.tile([128, HLOC, S], F32R, name="hoT1", tag="hoT1")
    # O(b0) is entirely ready at W4 start: two of its 32 pieces follow every
    # attention(b1) unit, filling the exp-latency PE slack uniformly
    opieces = [(qc, t, half)
               for qc in range(QC) for t in range(4) for half in range(2)]
    for u in range(HLOC * QC):
        h, qc = divmod(u, QC)
        i = HLOC + h
        if qc == 0 and h + 1 < HLOC and (i + 1) not in qk_tiles:
            qk_tiles[i + 1] = load_qk(i + 1)
            if h == 1:
                v_tiles[(1, 1)] = load_v(1, 1, nc.scalar)
        fin = attn_unit(i, qc)
        for p in (2 * u, 2 * u + 1):
            pqc, pt, phalf = opieces[p]
            o_piece(0, pqc, pt, phalf, hoTs[0], False)
        fin()

    # ---- W5: O-projection(b1) ----
    for qc in range(QC):
        o_block(1, qc, hoTs[1], three_way=True, last=(qc == QC - 1))

    psO.release()
    ost.release()
    hold1.release()
    for p in reversed(attn_ps):
        p.release()
    sm.release()
    hold.release()
    w3p.release()
    qkv.release()


# --------------------------------------------------------------------------
# legacy generic path (non-causal masks)
# --------------------------------------------------------------------------

def _build_legacy():
    import concourse.bacc as bacc
    import concourse.tile as tile
    from concourse import mybir

    F32 = mybir.dt.float32
    F32R = mybir.dt.float32r
    EXP = mybir.ActivationFunctionType.Exp

    nc = bacc.Bacc("TRN2", target_bir_lowering=False, debug=False, num_devices=NC)

    xt_d = nc.dram_tensor("xt", [DKT, 128, TOK], F32R, kind="ExternalInput")
    wq_d = nc.dram_tensor("wq", [128, DKT, 512], F32R, kind="ExternalInput")
    wk_d = nc.dram_tensor("wk", [128, DKT, 512], F32R, kind="ExternalInput")
    wv_d = nc.dram_tensor("wv", [128, DKT, 512], F32R, kind="ExternalInput")
    wo_d = nc.dram_tensor("wo", [128, HLOC, D], F32R, kind="ExternalInput")
    cs_d = nc.dram_tensor("cs", [128, S], F32, kind="ExternalInput")
    ss_d = nc.dram_tensor("ss", [128, S], F32, kind="ExternalInput")
    mk_d = nc.dram_tensor("maskf", [KT, 128, S], F32, kind="ExternalInput")
    out_d = nc.dram_tensor("out", [TOK, D], F32, kind="ExternalOutput")

    qdr = {(h, b): nc.dram_tensor(f"qdr{h}_{b}", [128, S], F32R)
           for h in range(HLOC) for b in range(B)}
    kdr = {(h, b): nc.dram_tensor(f"kdr{h}_{b}", [128, S], F32R)
           for h in range(HLOC) for b in range(B)}
    vdr = {b: nc.dram_tensor(f"vdr{b}", [S, 512], F32R) for b in range(B)}

    with tile.TileContext(nc) as tc:
        with tc.tile_pool(name="consts", bufs=1) as consts:
            ones_sb = consts.tile([128, 1], F32R)
            nc.vector.memset(ones_sb.bitcast(F32), 1.0)

            # Phase 1b: V projection
            with (
                tc.tile_pool(name="w2", bufs=1) as w2,
                tc.tile_pool(name="xt2", bufs=4) as xt2,
                tc.tile_pool(name="vcp", bufs=4) as vcp,
                tc.tile_pool(name="ps2", bufs=2, space="PSUM") as ps2,
            ):
                wv_sb = w2.tile([128, DKT, 512], F32R, tag="wv")
                for ch in range(CH):
                    b, s0 = ch // QC, (ch % QC) * 512
                    vps = [ps2.tile([128, 512], F32, name=f"vps{t}", tag=f"v{t}")
                           for t in range(4)]
                    for dk in range(DKT):
                        if ch == 0:
                            we = nc.scalar if dk % 2 == 0 else nc.sync
                            we.dma_start(out=wv_sb[:, dk, :], in_=wv_d.ap()[:, dk, :])
                        xt = xt2.tile([128, 512], F32R, name="xt", tag="xt")
                        nc.sync.dma_start(
                            out=xt, in_=xt_d.ap()[dk, :, ch * 512:(ch + 1) * 512]
                        )
                        for t in range(4):
                            nc.tensor.matmul(
                                vps[t], xt[:, t * 128:(t + 1) * 128], wv_sb[:, dk, :],
                                start=(dk == 0), stop=(dk == DKT - 1),
                            )
                    for t in range(4):
                        vc = vcp.tile([128, 512], F32R, tag="vc")
                        nc.vector.tensor_copy(vc, vps[t])
                        nc.gpsimd.dma_start(
                            out=vdr[b].ap()[s0 + t * 128:s0 + (t + 1) * 128, :],
                            in_=vc,
                        )

            # Phase 1a: Q,K projections + RoPE
            with (
                tc.tile_pool(name="w1", bufs=1) as w1,
                tc.tile_pool(name="xt1", bufs=4) as xt1,
                tc.tile_pool(name="rope", bufs=2) as rope,
                tc.tile_pool(name="ps1", bufs=1, space="PSUM") as ps1,
            ):
                wq_sb = w1.tile([128, DKT, 512], F32R, tag="wq")
                wk_sb = w1.tile([128, DKT, 512], F32R, tag="wk")
                for ch in range(CH):
                    b, s0 = ch // QC, (ch % QC) * 512
                    cs_sb = rope.tile([128, 512], F32, name="cs_c", tag="cs_c")
                    ss_sb = rope.tile([128, 512], F32, name="ss_c", tag="ss_c")
                    nc.scalar.dma_start(out=cs_sb, in_=cs_d.ap()[:, s0:s0 + 512])
                    nc.scalar.dma_start(out=ss_sb, in_=ss_d.ap()[:, s0:s0 + 512])
                    qps = [ps1.tile([128, 512], F32, name=f"qps{h}", tag=f"q{h}")
                           for h in range(HLOC)]
                    kps = [ps1.tile([128, 512], F32, name=f"kps{h}", tag=f"k{h}")
                           for h in range(HLOC)]
                    for dk in range(DKT):
                        if ch == 0:
                            we = nc.scalar if dk % 2 == 0 else nc.sync
                            wf = nc.sync if dk % 2 == 0 else nc.scalar
                            we.dma_start(out=wq_sb[:, dk, :], in_=wq_d.ap()[:, dk, :])
                            wf.dma_start(out=wk_sb[:, dk, :], in_=wk_d.ap()[:, dk, :])
                        xt = xt1.tile([128, 512], F32R, name="xt", tag="xt")
                        nc.sync.dma_start(
                            out=xt, in_=xt_d.ap()[dk, :, ch * 512:(ch + 1) * 512]
                        )
                        for h in range(HLOC):
                            nc.tensor.matmul(
                                qps[h], wq_sb[:, dk, h * 128:(h + 1) * 128], xt,
                                start=(dk == 0), stop=(dk == DKT - 1),
                            )
                        for h in range(HLOC):
                            nc.tensor.matmul(
                                kps[h], wk_sb[:, dk, h * 128:(h + 1) * 128], xt,
                                start=(dk == 0), stop=(dk == DKT - 1),
                            )
                    work = []
                    for h in range(HLOC):
                        for ps, dst in ((qps[h], qdr), (kps[h], kdr)):
                            pc = rope.tile([128, 512], F32, name="pc", tag="pc",
                                           bufs=4)
                            t1 = rope.tile([128, 512], F32, name="t1", tag="t1",
                                           bufs=8)
                            s1 = rope.tile([128, 512], F32, name="s1", tag="s1",
                                           bufs=2)
                            s1w = rope.tile([128, 512], F32, name="s1w", tag="s1w",
                                            bufs=8)
                            nc.vector.tensor_copy(pc, ps)
                            nc.vector.tensor_mul(t1, pc, cs_sb)
                            nc.vector.tensor_mul(s1, pc, ss_sb)
                            nc.scalar.dma_start(out=s1w[0:64, :], in_=s1[64:128, :])
                            nc.scalar.dma_start(out=s1w[64:128, :], in_=s1[0:64, :])
                            work.append((h, dst, t1, s1w))
                    for h, dst, t1, s1w in work:
                        rr = rope.tile([128, 512], F32R, name="rr", tag="rr", bufs=2)
                        nc.vector.tensor_add(rr, t1, s1w)
                        nc.scalar.dma_start(out=dst[(h, b)].ap()[:, s0:s0 + 512],
                                            in_=rr)

            # Phases 2+3
            hbs = [(b, h) for b in range(B) for h in range(HLOC)]
            with (
                tc.tile_pool(name="qkv", bufs=2) as qkv,
                tc.tile_pool(name="hold", bufs=1) as hold,
                tc.tile_pool(name="smp", bufs=2) as smp,
                tc.tile_pool(name="ps3", bufs=1, space="PSUM") as ps3,
                tc.tile_pool(name="ps4", bufs=3, space="PSUM") as ps4,
            ):
                def load_hb(i):
                    b, h = hbs[i]
                    qT = qkv.tile([128, S], F32R, name=f"qT_{i}", tag="qT")
                    kT = qkv.tile([128, S], F32R, name=f"kT_{i}", tag="kT")
                    vT = qkv.tile([128, KT, 128], F32R, name=f"vT_{i}", tag="vT")
                    vsrc = vdr[b].ap()[:, h * 128:(h + 1) * 128].rearrange(
                        "(n p) d -> p n d", p=128
                    )
                    for j in range(QC):
                        sl = slice(j * 512, (j + 1) * 512)
                        nc.sync.dma_start(out=qT[:, sl], in_=qdr[(h, b)].ap()[:, sl])
                        nc.sync.dma_start(out=kT[:, sl], in_=kdr[(h, b)].ap()[:, sl])
                        nc.sync.dma_start(
                            out=vT[:, j * 4:(j + 1) * 4, :],
                            in_=vsrc[:, j * 4:(j + 1) * 4, :],
                        )
                    return qT, kT, vT

                tiles = {0: load_hb(0)}
                wo_sb = hold.tile([128, HLOC, D], F32R, tag="wo")
                for h in range(HLOC):
                    nc.scalar.dma_start(out=wo_sb[:, h, :], in_=wo_d.ap()[:, h, :])

                hoTs = {}
                for i, (b, h) in enumerate(hbs):
                    if h == 0:
                        hoTs[b] = hold.tile([128, HLOC, S], F32R,
                                            name=f"hoT_{b}", tag=f"hoT{b}")
                    hoT = hoTs[b]
                    if i + 1 < len(hbs):
                        tiles[i + 1] = load_hb(i + 1)
                    qT, kT, vT = tiles.pop(i)
                    for qc in range(QC):
                        qs = qc * 512
                        sums = ps3.tile([1, 512], F32, name="sums", tag="sums")
                        hops = ps3.tile([128, 512], F32, name="hops", tag="hops")
                        for kt in range(KT):
                            st = ps4.tile([128, 512], F32, name="st", tag="st")
                            nc.tensor.matmul(
                                st, kT[:, kt * 128:(kt + 1) * 128],
                                qT[:, qs:qs + 512], start=True, stop=True,
                            )
                            mkt = smp.tile([128, 512], F32, name="mkt", tag="mkt")
                            nc.sync.dma_start(out=mkt, in_=mk_d.ap()[kt, :, qs:qs + 512])
                            nc.vector.tensor_add(st, st, mkt)
                            ex = smp.tile([128, 512], F32R, name="ex", tag="ex",
                                          bufs=4)
                            nc.scalar.activation(ex, st, EXP, scale=ISQRT)
                            nc.tensor.matmul(sums, ones_sb, ex, start=(kt == 0),
                                             stop=(kt == KT - 1))
                            nc.tensor.matmul(hops, vT[:, kt, :], ex, start=(kt == 0),
                                             stop=(kt == KT - 1))
                        recip = smp.tile([1, 512], F32, name="recip", tag="recip")
                        nc.vector.reciprocal(recip, sums)
                        bc = smp.tile([128, 512], F32, name="bc", tag="bc")
                        nc.gpsimd.partition_broadcast(bc, recip)
                        nc.vector.tensor_mul(hoT[:, h, qs:qs + 512], hops, bc)

                for b in range(B):
                    with (
                        tc.tile_pool(name=f"oc{b}", bufs=3) as ocp,
                        tc.tile_pool(name=f"ps5{b}", bufs=3, space="PSUM") as ps5,
                    ):
                        for t in range(S // 128):
                            for oc in range(D // 512):
                                ops = ps5.tile([128, 512], F32, name="ops", tag="ops")
                                for h in range(HLOC):
                                    nc.tensor.matmul(
                                        ops, hoTs[b][:, h, t * 128:(t + 1) * 128],
                                        wo_sb[:, h, oc * 512:(oc + 1) * 512],
                                        start=(h == 0), stop=(h == HLOC - 1),
                                    )
                                ot = ocp.tile([128, 512], F32, name="ot", tag="ot")
                                nc.vector.tensor_copy(ot, ops)
                                nc.scalar.dma_start(
                                    out=out_d.ap()[
                                        b * S + t * 128:b * S + (t + 1) * 128,
                                        oc * 512:(oc + 1) * 512,
                                    ],
                                    in_=ot,
                                )

    nc.compile()
    return nc


def _get_nc(causal: bool):
    if causal not in _CACHE:
        _CACHE[causal] = _build_causal() if causal else _build_legacy()
    return _CACHE[causal]


def _host_prep(x, wq, wk, wv, wo, freqs_cos, freqs_sin, mask):
    """Build per-core input maps."""
    x2 = np.ascontiguousarray(x.reshape(TOK, D).T)          # [D, TOK]
    xt = x2.reshape(DKT, 128, TOK)

    cs = np.concatenate([freqs_cos.T, freqs_cos.T], axis=0).astype(np.float32)
    ss = np.concatenate([freqs_sin.T, -freqs_sin.T], axis=0).astype(np.float32)

    m2 = np.asarray(mask, dtype=np.float32).reshape(S, S)
    tril = np.tril(np.ones((S, S), dtype=bool))
    causal = bool(np.all(m2[tril] == 0.0) and np.all(m2[~tril] <= -1e8))
    if causal:
        mk = np.ascontiguousarray(m2[:128, :128].T)         # [k,q] triangle
    else:
        mk = np.ascontiguousarray(m2.T.reshape(KT, 128, S))

    # per-head column permutation: evens then odds (RoPE rotate-half form)
    perm = np.concatenate([np.arange(0, HD, 2), np.arange(1, HD, 2)])

    xt16 = xt.astype(np.float16)

    in_maps = []
    for c in range(NC):
        cols = np.concatenate(
            [(4 * c + h) * HD + perm for h in range(HLOC)]
        )
        wq_c = np.ascontiguousarray(
            wq[:, cols].reshape(DKT, 128, 512).transpose(1, 0, 2)
        )
        wk_c = np.ascontiguousarray(
            wk[:, cols].reshape(DKT, 128, 512).transpose(1, 0, 2)
        )
        vcols = np.arange(4 * c * HD, 4 * (c + 1) * HD)
        wv_c = np.ascontiguousarray(
            wv[:, vcols].reshape(DKT, 128, 512).transpose(1, 0, 2)
        )
        wo_c = np.ascontiguousarray(
            wo[vcols, :].reshape(HLOC, 128, D).transpose(1, 0, 2)
        )
        if causal:
            wq_c = wq_c.astype(np.float16)
            wk_c = wk_c.astype(np.float16)
            wv_c = wv_c.astype(np.float16)
            m = {
                "xt16": xt16, "wq": wq_c, "wk": wk_c, "wv": wv_c,
                "wo": wo_c, "cs": cs, "ss": ss, "mtri": mk,
            }
        else:
            m = {
                "xt": xt, "wq": wq_c, "wk": wk_c, "wv": wv_c, "wo": wo_c,
                "cs": cs, "ss": ss, "maskf": mk,
            }
        in_maps.append(m)
    return in_maps, causal


def kernel(x, wq, wk, wv, wo, freqs_cos, freqs_sin, mask, **_unused):
    from concourse.bass_utils import run_bass_kernel_spmd

    x = np.asarray(x, dtype=np.float32)
    wq = np.asarray(wq, dtype=np.float32)
    wk = np.asarray(wk, dtype=np.float32)
    wv = np.asarray(wv, dtype=np.float32)
    wo = np.asarray(wo, dtype=np.float32)
    freqs_cos = np.asarray(freqs_cos, dtype=np.float32)
    freqs_sin = np.asarray(freqs_sin, dtype=np.float32)

    in_maps, causal = _host_prep(x, wq, wk, wv, wo, freqs_cos, freqs_sin, mask)
    nc = _get_nc(causal)
    res = run_bass_kernel_spmd(nc, in_maps, list(range(NC)))
    out = res.results[0]["out"].astype(np.float32)
    for c in range(1, NC):
        out = out + res.results[c]["out"].astype(np.float32)
    return out.reshape(B, S, D).astype(np.float32)

